# revision 78
# baseline (speedup 1.0000x reference)
# Trainium2 Bass kernel for nn_Encoder_81509889343552 — spatially sharded v2.
#
# Each image (B=4) is split top/bottom across a core pair (8 cores total).
# Uniform SPMD spans via exact doubling: per-core local row spans
#   h0=288, h1=144, h2=72, h3=36, h4=18, g0=36, g1=72, g2=144, g3=288, hf=288.
# InstanceNorm stats are exact: per-strip bn_stats entries are weighted by a
# host 0/1 ownership vector (each core owns its half of every layer), then a
# tiny per-layer AllReduce of (mean, var+mean^2) across the pair combines the
# halves. Junk rows near the interior boundary (from zero-pad instead of real
# neighbor data) are excluded from ownership by construction.
#
# bf16 activations/weights (f32 stats + PSUM), K-packed L1 (K=96), pair-packed
# h0 (d0 gets K=128+K=64 taps), M-packed u3 (row pairs in PE columns), fused
# masked segment mean (no hf DRAM roundtrip; tanh(mask*(x+b)) = mask*tanh(x+b)
# for a binary mask). Host combines the pair partial sums and scatters.
import sys

sys.path.insert(0, "/opt/trn_rl_repo")

import contextlib

import numpy as np
import ml_dtypes

import concourse.bass as bass
import concourse.bacc as bacc
import concourse.tile as tile
from concourse import mybir
from concourse.bass_utils import run_bass_kernel_spmd

F32 = mybir.dt.float32
F32R = mybir.dt.float32r
BF16 = mybir.dt.bfloat16
AF = mybir.ActivationFunctionType
ALU = mybir.AluOpType
BFH = ml_dtypes.bfloat16
F8 = mybir.dt.float8e4
F8H = ml_dtypes.float8_e4m3
DRM = mybir.MatmulPerfMode.DoubleRow

B, H, W = 4, 512, 512
EPS = 1e-5
P = 128
RG = [[0, 1], [2, 3], [4, 5], [6, 7]]

# local span / bottom base / owned-cutoff table (top cut = owned end,
# bot cut = owned start, in local rows)
SPAN = {
    "h0": (288, 224, 256, 32), "h1": (144, 112, 128, 16),
    "h2": (72, 56, 64, 8),     "h3": (36, 28, 32, 4),
    "h4": (18, 14, 16, 2),     "g0": (36, 28, 32, 4),
    "g1": (72, 56, 64, 8),     "g2": (144, 112, 128, 16),
    "g3": (288, 224, 256, 32),
}


def _ap(base, extra_off, dims):
    return bass.AP(
        tensor=base.tensor,
        offset=base.offset + extra_off,
        ap=[list(base.ap[0])] + [list(d) for d in dims],
    )


def _dap(handle, off, dims):
    return bass.AP(tensor=handle, offset=off, ap=[list(d) for d in dims])


# ---------------------------------------------------------------------------
# Stats-entry enumeration (shared between host weight gen and device build).
# Each entry is (row_lo, row_hi) half-open in local layer rows; the device
# emits bn_stats in exactly this order per channel-block.
# ---------------------------------------------------------------------------

def entries_L1():
    return [(2 * k, 2 * k + 2) for k in range(144)]


def entries_d0():
    return [(2 * e, 2 * e + 2) for e in range(72)]


def entries_down(nstrip, nchunk, nrc):
    out = []
    for s in range(nstrip):
        for c in range(nchunk):
            lo = s * nchunk * nrc + c * nrc
            out.append((lo, lo + nrc))
    return out


def slices_d3(chunk):
    # chunk 0 = rows 0..8, chunk 1 = rows 9..17; cutoffs at rows {2, 16}
    return [(0, 2), (2, 9)] if chunk == 0 else [(0, 7), (7, 9)]


def entries_d3():
    return [(0, 2), (2, 9), (9, 16), (16, 18)]


# --- up-layer su/slice schedules -------------------------------------------
# UP_SCHED[name] = (nstrip, subs_per_strip, slices(s, su) -> [(klo,khi)])
def _u0_slices(s, su):
    return [(0, 2), (2, 9)] if su == 0 else [(0, 7), (7, 9)]


def _u1_slices(s, su):
    if s == 0:
        return [(0, 4), (4, 8)] if su == 0 else \
            ([(0, 8)] if su == 1 else [(0, 2)])
    return [(0, 6), (6, 8)] if su == 1 else \
        ([(0, 8)] if su == 0 else [(0, 2)])


def _u2_slices(s, su):
    if s == 3 and su == 2:
        return [(0, 2), (2, 4)]
    return [(0, 4)] if su < 4 else [(0, 2)]


UP_SCHED = {
    "g0": (1, [9, 9], _u0_slices),
    "g1": (2, [8, 8, 2], _u1_slices),
    "g2": (4, [4, 4, 4, 4, 2], _u2_slices),
}


def up_entries_and_index(name, Sout):
    nstrip, subs, slfn = UP_SCHED[name]
    nro = Sout // nstrip
    ents, idx = [], {}
    for s in range(nstrip):
        y0 = s * nro
        for a in range(2):
            for b_ in range(2):
                for su in range(len(subs)):
                    k0 = sum(subs[:su])
                    for ei, (klo, khi) in enumerate(slfn(s, su)):
                        idx[(s, a, b_, su, ei)] = len(ents)
                        ents.append((y0 + a + 2 * (k0 + klo),
                                     y0 + a + 2 * (k0 + khi - 1) + 1))
    return ents, idx


ENT_G0, IDX_G0 = up_entries_and_index("g0", 36)
ENT_G1, IDX_G1 = up_entries_and_index("g1", 72)
ENT_G2, IDX_G2 = up_entries_and_index("g2", 144)


def entries_u3():
    # 6 strips x 12 blocks x 2 (pb0, pb1); block = 2 out-row pairs = 4 rows
    out = []
    for s in range(6):
        for blk in range(12):
            q0 = s * 24 + blk * 2
            out.append((2 * q0, 2 * q0 + 4))
            out.append((2 * q0, 2 * q0 + 4))
    return out


LAYER_ENTRIES = {
    "h0": entries_L1(), "h1": entries_d0(),
    "h2": entries_down(6, 3, 4), "h3": entries_down(3, 3, 4),
    "h4": entries_d3(), "g0": ENT_G0, "g1": ENT_G1,
    "g2": ENT_G2, "g3": entries_u3(),
}
LAYER_ORDER = ["h0", "h1", "h2", "h3", "h4", "g0", "g1", "g2", "g3"]
LAYER_CBO = {"h0": 1, "h1": 1, "h2": 2, "h3": 4, "h4": 8,
             "g0": 4, "g1": 2, "g2": 1, "g3": 1}


def statw_vector(half):
    """Concatenated per-entry 0/1 weights (x6 fields, replicated per
    channel-block) for this core half."""
    vals = []
    offs = {}
    for name in LAYER_ORDER:
        S, bbase, cut_top, cut_bot = SPAN[name]
        offs[name] = len(vals)
        lw = []
        for (lo, hi) in LAYER_ENTRIES[name]:
            if half == 0:
                w = 1.0 if hi <= cut_top else 0.0
                assert hi <= cut_top or lo >= cut_top, (name, lo, hi)
            else:
                w = 1.0 if lo >= cut_bot else 0.0
                assert lo >= cut_bot or hi <= cut_bot, (name, lo, hi)
            lw.extend([w] * 6)
        vals.extend(lw * LAYER_CBO[name])
    return np.asarray(vals, np.float32), offs


STATW_TOP, STATW_OFFS = statw_vector(0)
STATW_BOT, _ = statw_vector(1)
NSTATW = len(STATW_TOP)


# ---------------------------------------------------------------------------
# Host-side weight preprocessing (all lhsT blobs in SBUF layout, bf16)
# ---------------------------------------------------------------------------

def prep_weights(inp):
    w = {}
    # L1: K = (ci, dy, l4) = 96; M = r*64+co; matmul d in {0,1}: kx = 4d + l
    w0 = np.asarray(inp["w0"], np.float32)  # [64, 3, 7, 7]
    w1 = np.zeros((96, 2, 128), np.float32)
    for ci in range(3):
        for dy in range(8):
            for l in range(4):
                p = ci * 32 + dy * 4 + l
                for d in range(2):
                    kx = 4 * d + l
                    if kx > 6:
                        continue
                    for r in range(2):
                        ky = dy - r
                        if 0 <= ky <= 6:
                            w1[p, d, r * 64:(r + 1) * 64] = w0[:, ci, ky, kx]
    w["w1"] = w1.astype(BFH)

    # d0 (pair-packed h0): K128 tap (pair y): rows (j,c): ky=1+j;
    # K64 tap (pair y-1, j=1 partitions 64..127): ky=0.
    dw0 = np.asarray(inp["dw0"], np.float32)  # [128, 64, 3, 3]
    wd0a = np.zeros((128, 3, 128), np.float32)
    wd0b = np.zeros((128, 3, 128), np.float32)
    for dx in range(3):
        for j in range(2):
            for c in range(64):
                wd0a[j * 64 + c, dx, :] = dw0[:, c, 1 + j, dx]
        for c in range(64):
            wd0b[64 + c, dx, :] = dw0[:, c, 0, dx]
    w["wd0a"] = wd0a.astype(BFH)
    w["wd0b"] = wd0b.astype(BFH)

    # d1..d3: [cbo, K=128, cbi, 3, 3, M=128] (k-major per m-block)
    c = 128
    for i in (1, 2, 3):
        dw = np.asarray(inp[f"dw{i}"], np.float32)  # [2c, c, 3, 3]
        cbo, cbi = (2 * c) // P, c // P
        blob = np.zeros((cbo, P, cbi, 3, 3, P), np.float32)
        for m in range(cbo):
            for cb in range(cbi):
                for dy in range(3):
                    for dx in range(3):
                        blob[m, :, cb, dy, dx, :] = \
                            dw[m * P:(m + 1) * P, cb * P:(cb + 1) * P, dy, dx].T
        w[f"wd{i}"] = blob.astype(BFH if i == 1 else F8H)
        c *= 2

    # u0..u2 (torch convT layout uw [Cin, Cout, 3, 3]):
    # [cbo, K=128, cbi, 3, 3, Mo]
    for i in (0, 1, 2):
        uw = np.asarray(inp[f"uw{i}"], np.float32)
        Cin_, Cout_ = uw.shape[0], uw.shape[1]
        cbi, cbo, Mo = Cin_ // P, max(Cout_ // P, 1), min(Cout_, P)
        blob = np.zeros((cbo, P, cbi, 3, 3, Mo), np.float32)
        for m in range(cbo):
            for cb in range(cbi):
                for ky in range(3):
                    for kx in range(3):
                        blob[m, :, cb, ky, kx, :] = \
                            uw[cb * P:(cb + 1) * P, m * Mo:(m + 1) * Mo, ky, kx]
        w[f"wu{i}"] = blob.astype(F8H)

    # u3 M-packed: psum partition q = j*64 + c (j = out row parity).
    # T1 (b0, rhs i=q,  col p):  j0: (ky1,kx1); j1: (ky2,kx1)
    # T2 (b0, rhs i=q+1,col p):  j1: (ky0,kx1)   [M 64..127]
    # T3 (b1, rhs i=q,  col p):  j0: (ky1,kx2); j1: (ky2,kx2)
    # T4 (b1, rhs i=q,  col p+1):j0: (ky1,kx0); j1: (ky2,kx0)
    # T5 (b1, rhs i=q+1,col p):  j1: (ky0,kx2)
    # T6 (b1, rhs i=q+1,col p+1):j1: (ky0,kx0)
    uw3 = np.asarray(inp["uw3"], np.float32)  # [128, 64, 3, 3]
    wa = np.zeros((128, 3, 128), np.float32)  # T1, T3, T4
    wb = np.zeros((128, 3, 64), np.float32)   # T2, T5, T6
    for t, (ky0_, kx0_, ky1_, kx1_) in enumerate(
            [(1, 1, 2, 1), (1, 2, 2, 2), (1, 0, 2, 0)]):
        wa[:, t, 0:64] = uw3[:, :, ky0_, kx0_]
        wa[:, t, 64:128] = uw3[:, :, ky1_, kx1_]
    for t, (ky_, kx_) in enumerate([(0, 1), (0, 2), (0, 0)]):
        wb[:, t, :] = uw3[:, :, ky_, kx_]
    w["wu3a"] = wa.astype(BFH)
    w["wu3b"] = wb.astype(BFH)

    # final conv stage A: K = (j, c) over 4 g3-pairs; slab pair t holds padded
    # rows 6k+2t+j for strip y0=6k; out row y0+r reads padded y0+1+ky'
    # (pad offset 4 at top => padded row == local row + 4... see g3 layout):
    # tap ky' = 2t + j - r - 1; M = r*21 + dx*3 + co.
    wf = np.asarray(inp["wf"], np.float32)  # [3, 64, 7, 7]
    wfA = np.zeros((128, 7, 126), np.float32)
    for t in range(7):
        for j in range(2):
            for r in range(6):
                ky = 2 * t + j - r - 1
                if 0 <= ky <= 6:
                    for dx in range(7):
                        for co in range(3):
                            wfA[j * 64:(j + 1) * 64, t, r * 21 + dx * 3 + co] = \
                                wf[co, :, ky, dx]
    w["wfA"] = wfA.astype(BFH)
    wfS = np.zeros((126, 7, 18), np.float32)
    for dx in range(7):
        for r in range(6):
            for co in range(3):
                wfS[r * 21 + dx * 3 + co, dx, r * 3 + co] = 1.0
    w["wfS"] = wfS.astype(BFH)
    bf = np.asarray(inp["bf"], np.float32)
    w["bfv"] = np.tile(bf, 6).reshape(18, 1).astype(np.float32)

    # j-fold (average partitions c and c+64) for h0 / g3 stats
    wfold = np.zeros((128, 64), np.float32)
    for j in range(2):
        for c_ in range(64):
            wfold[j * 64 + c_, c_] = 0.5
    w["wfold"] = wfold
    return w


def prep_core_inputs(x_img, inst_img, wblobs, half):
    """Per-core inputs: xrep (K-packed padded x slice), maskrep, statw."""
    xpad = np.pad(np.asarray(x_img, np.float32), ((0, 0), (3, 3), (3, 3)),
                  mode="reflect")  # [3, 518, 518]
    r0 = 0 if half == 0 else 224
    xrep = np.zeros((96, 288, 518), np.float32)
    for ci in range(3):
        for dy in range(8):
            for l in range(4):
                p = ci * 32 + dy * 4 + l
                hi = min(r0 + dy + 288, 518)
                rows = xpad[ci, r0 + dy:hi, :]
                xrep[p, :hi - (r0 + dy), :518 - l] = rows[:, l:]
    mask = (np.asarray(inst_img) == 1).astype(np.float32)  # [512, 512]
    base = 0 if half == 0 else 224
    maskrep = np.zeros((18, 48, 2, 256), np.float32)
    for s in range(48):
        for r in range(6):
            y = 6 * s + r
            gy = base + y
            owned = (y < 256) if half == 0 else (y >= 32)
            if owned:
                row = mask[gy]
                for co in range(3):
                    maskrep[r * 3 + co, s, 0, :] = row[:256]
                    maskrep[r * 3 + co, s, 1, :] = row[256:]
    m = {
        "xrep": xrep.astype(F8H),
        "maskrep": maskrep.reshape(18, 48 * 2 * 256),
        "statw": STATW_TOP if half == 0 else STATW_BOT,
    }
    m.update(wblobs)
    return m


# ---------------------------------------------------------------------------
# Device kernel
# ---------------------------------------------------------------------------

def build_kernel(debug=False):
    nc = bacc.Bacc(None, target_bir_lowering=False, num_devices=8)

    xrep = nc.dram_tensor("xrep", [96, 288, 518], F8, kind="ExternalInput")
    maskrep = nc.dram_tensor("maskrep", [18, 48 * 2 * 256], F32,
                             kind="ExternalInput")
    statw = nc.dram_tensor("statw", [NSTATW], F32, kind="ExternalInput")
    w1 = nc.dram_tensor("w1", [96, 2, 128], BF16, kind="ExternalInput")
    wd0a = nc.dram_tensor("wd0a", [128, 3, 128], BF16, kind="ExternalInput")
    wd0b = nc.dram_tensor("wd0b", [128, 3, 128], BF16, kind="ExternalInput")
    wd = {}
    c = 128
    for i in (1, 2, 3):
        cbo, cbi = (2 * c) // P, c // P
        wd[i] = nc.dram_tensor(f"wd{i}", [cbo, P, cbi, 3, 3, P],
                               BF16 if i == 1 else F8,
                               kind="ExternalInput")
        c *= 2
    wu = {}
    c = 1024
    for i in (0, 1, 2):
        cbi, cbo, Mo = c // P, max((c // 2) // P, 1), min(c // 2, P)
        wu[i] = nc.dram_tensor(f"wu{i}", [cbo, P, cbi, 3, 3, Mo], F8,
                               kind="ExternalInput")
        c //= 2
    wu3a = nc.dram_tensor("wu3a", [128, 3, 128], BF16, kind="ExternalInput")
    wu3b = nc.dram_tensor("wu3b", [128, 3, 64], BF16, kind="ExternalInput")
    wfA = nc.dram_tensor("wfA", [128, 7, 126], BF16, kind="ExternalInput")
    wfS = nc.dram_tensor("wfS", [126, 7, 18], BF16, kind="ExternalInput")
    bfv = nc.dram_tensor("bfv", [18, 1], F32, kind="ExternalInput")
    wfold = nc.dram_tensor("wfold", [128, 64], F32, kind="ExternalInput")

    h0 = nc.dram_tensor("h0", [128, 144, 512], F8)  # pair-packed (j,c)
    h1 = nc.dram_tensor("h1", [1, 128, 144, 256], F8)
    h2 = nc.dram_tensor("h2", [2, 128, 72, 128], F8)
    h3 = nc.dram_tensor("h3", [4, 128, 36, 64], F8)
    h4 = nc.dram_tensor("h4", [8, 128, 18, 32], F8)
    g0 = nc.dram_tensor("g0", [4, 128, 36, 64], F8)
    g1 = nc.dram_tensor("g1", [2, 128, 72, 128], F8)
    g2 = nc.dram_tensor("g2", [1, 128, 144, 256], F8)
    g3 = nc.dram_tensor("g3", [128, 148, 518], F8)  # pair-packed, pad4 top
    # per-layer stats scratch in DRAM + allreduced copy
    CT = {"h0": 64, "h1": 128, "h2": 256, "h3": 512, "h4": 1024,
          "g0": 512, "g1": 256, "g2": 128, "g3": 64}
    stat_l = {k: nc.dram_tensor(f"stl_{k}", [v, 2], F32)
              for k, v in CT.items()}
    stat_r = {k: nc.dram_tensor(f"str_{k}", [2 * v, 2], F32)
              for k, v in CT.items()}
    osum = nc.dram_tensor("osum", [18, 1], F32, kind="ExternalOutput")

    dbg = {}
    if debug:
        for nm, sh in [("h0", [128, 144 * 512]), ("h1", [128, 144 * 256]),
                       ("h2", [256, 72 * 128]), ("h3", [512, 36 * 64]),
                       ("h4", [1024, 18 * 32]), ("g0", [512, 36 * 64]),
                       ("g1", [256, 72 * 128]), ("g2", [128, 144 * 256]),
                       ("g3", [128, 148 * 518])]:
            dbg[nm] = nc.dram_tensor("dbg_" + nm, sh, BF16,
                                     kind="ExternalOutput")
        dbg["st"] = nc.dram_tensor("dbg_st", [128, 2 * 9], F32,
                                   kind="ExternalOutput")
        dbg["sr"] = nc.dram_tensor("dbg_sr", [sum(CT.values()), 2], F32,
                                   kind="ExternalOutput")
        dbg["sl"] = nc.dram_tensor("dbg_sl", [sum(CT.values()), 2], F32,
                                   kind="ExternalOutput")
        dbg["stt1"] = nc.dram_tensor("dbg_stt1", [128, 144 * 6], F32,
                                     kind="ExternalOutput")

    with tile.TileContext(nc) as tc, contextlib.ExitStack() as ctx:
        sb = ctx.enter_context(tc.tile_pool(name="sb", bufs=3))
        osl = ctx.enter_context(tc.tile_pool(name="osl", bufs=2))
        wsm = ctx.enter_context(tc.tile_pool(name="wsm", bufs=1))
        wpm = ctx.enter_context(tc.tile_pool(name="wpm", bufs=2))
        nrm = ctx.enter_context(tc.tile_pool(name="nrm", bufs=1))
        stp = ctx.enter_context(tc.tile_pool(name="stp", bufs=1))
        ps = ctx.enter_context(tc.tile_pool(name="ps", bufs=3, space="PSUM"))
        psf = ctx.enter_context(tc.tile_pool(name="psf", bufs=2, space="PSUM"))

        eps_t = nrm.tile([P, 1], F32, name="eps_t")
        nc.vector.memset(eps_t, EPS)
        wfoldt = nrm.tile([P, 64], F32, name="wfoldt")
        nc.sync.dma_start(out=wfoldt, in_=wfold[:, :])

        # broadcast per-entry stat weights once: [128, NSTATW]
        # (NSTATW ~ 3.5k floats -> 14KB/partition; fine)
        wst_t = nrm.tile([P, NSTATW], F32, name="wst_t")
        nc.gpsimd.dma_start(out=wst_t,
                            in_=_dap(statw, 0, [[0, P], [1, NSTATW]]))

        st_tiles = {}
        HOLD = {}

        def layer_stats(name, stt, cbo, nent, fold=False):
            """stt [128, cbo, nent, 6] -> list of [128, 2] (scale, bias) APs
            per channel block. Weighted raw sums (NaN-proof, exact), batched
            over channel blocks; pairwise AllReduce of (mean, E)."""
            off = STATW_OFFS[name]
            n6 = cbo * nent * 6
            wl = {"h0": 512, "h1": 256, "h2": 128, "h3": 64, "h4": 32,
                  "g0": 64, "g1": 128, "g2": 256, "g3": 512}[name]
            npart = (wl // 2) * wl // (2 if fold else 1)
            tw = stp.tile([P, cbo, nent, 6], F32, name=f"tw_{name}", tag="tw")
            nc.vector.tensor_mul(
                out=tw.rearrange("p a b c -> p (a b c)"),
                in0=stt.rearrange("p a b c -> p (a b c)"),
                in1=wst_t[:, off:off + n6])
            cm = stp.tile([P, cbo, nent, 2], F32, name=f"cm_{name}", tag="cm")
            nc.vector.tensor_mul(
                out=cm.rearrange("p a b c -> p (a b c)"),
                in0=_ap(tw[:, 0, 0, 0], 0, [[6, cbo * nent], [3, 2]]),
                in1=_ap(tw[:, 0, 0, 0], 1, [[6, cbo * nent], [3, 2]]))
            e2 = stp.tile([P, cbo, nent, 2], F32, name=f"e2_{name}", tag="e2")
            nc.vector.tensor_mul(
                out=e2.rearrange("p a b c -> p (a b c)"),
                in0=cm.rearrange("p a b c -> p (a b c)"),
                in1=_ap(tw[:, 0, 0, 0], 1, [[6, cbo * nent], [3, 2]]))
            nc.vector.tensor_add(
                out=e2.rearrange("p a b c -> p (a b c)"),
                in0=e2.rearrange("p a b c -> p (a b c)"),
                in1=_ap(tw[:, 0, 0, 0], 2, [[6, cbo * nent], [3, 2]]))
            s1 = stp.tile([P, cbo], F32, name=f"s1_{name}", tag="s1")
            nc.vector.tensor_reduce(out=s1,
                                    in_=cm.rearrange("p a b c -> p a (b c)"),
                                    op=ALU.add, axis=mybir.AxisListType.X)
            s2 = stp.tile([P, cbo], F32, name=f"s2_{name}", tag="s2")
            nc.vector.tensor_reduce(out=s2,
                                    in_=e2.rearrange("p a b c -> p a (b c)"),
                                    op=ALU.add, axis=mybir.AxisListType.X)
            me = stp.tile([P, cbo, 2], F32, name=f"me_{name}", tag="me")
            nc.vector.tensor_scalar(out=me[:, :, 0:1], in0=s1,
                                    scalar1=1.0 / npart, scalar2=None,
                                    op0=ALU.mult)
            nc.vector.tensor_scalar(out=me[:, :, 1:2], in0=s2,
                                    scalar1=1.0 / npart, scalar2=None,
                                    op0=ALU.mult)
            if fold:
                pm = psf.tile([64, 2], F32, name=f"pm_{name}", tag="mini",
                              bufs=1)
                nc.tensor.matmul(pm, wfoldt, me[:, 0, :], start=True,
                                 stop=True)
                mef = stp.tile([64, 2], F32, name=f"mef_{name}", tag="mef")
                nc.scalar.activation(out=mef, in_=pm, func=AF.Copy)
                nc.gpsimd.dma_start(out=stat_l[name][0:64, :], in_=mef)
            else:
                nc.gpsimd.dma_start(
                    out=_dap(stat_l[name], 0, [[2, P], [256, cbo], [1, 2]]),
                    in_=me)
            nc.gpsimd.collective_compute(
                "AllGather", ALU.bypass, RG,
                ins=[stat_l[name][:, :]], outs=[stat_r[name][:, :]])
            # result loads go on the scalar queue so slab loads on sync are
            # not head-of-line blocked behind the collective
            ct_ = CT[name]
            lr = stp.tile([P, cbo, 2], F32, name=f"lr_{name}", tag="lr")
            lrb = stp.tile([P, cbo, 2], F32, name=f"lrb_{name}", tag="lrb")
            if fold:
                nc.scalar.dma_start(out=lr[0:64, 0, :],
                                    in_=stat_r[name][0:64, :])
                nc.sync.dma_start(out=lr[64:128, 0, :],
                                    in_=stat_r[name][0:64, :])
                nc.scalar.dma_start(out=lrb[0:64, 0, :],
                                    in_=stat_r[name][64:128, :])
                nc.gpsimd.dma_start(out=lrb[64:128, 0, :],
                                    in_=stat_r[name][64:128, :])
            else:
                nc.scalar.dma_start(
                    out=lr,
                    in_=_dap(stat_r[name], 0, [[2, P], [256, cbo], [1, 2]]))
                nc.gpsimd.dma_start(
                    out=lrb,
                    in_=_dap(stat_r[name], 2 * ct_,
                             [[2, P], [256, cbo], [1, 2]]))
            nc.vector.tensor_add(out=lr.rearrange("p a b -> p (a b)"),
                                 in0=lr.rearrange("p a b -> p (a b)"),
                                 in1=lrb.rearrange("p a b -> p (a b)"))
            t0 = stp.tile([P, cbo, 2], F32, name=f"t0_{name}", tag="t0")
            nc.vector.tensor_scalar(out=t0.rearrange("p a b -> p (a b)"),
                                    in0=lr.rearrange("p a b -> p (a b)"),
                                    scalar1=0.5, scalar2=None, op0=ALU.mult)
            mview = _ap(t0[:, 0, 0], 0, [[2, cbo]])
            eview = _ap(t0[:, 0, 0], 1, [[2, cbo]])
            var = stp.tile([P, cbo], F32, name=f"var_{name}", tag="var")
            nc.vector.tensor_mul(out=var, in0=mview, in1=mview)
            nc.vector.tensor_sub(out=var, in0=eview, in1=var)
            sd = stp.tile([P, cbo], F32, name=f"sd_{name}", tag="sd")
            nc.scalar.activation(out=sd, in_=var, func=AF.Sqrt,
                                 bias=eps_t, scale=1.0)
            stD = nrm.tile([P, cbo, 2], F32, name=f"st_{name}",
                           tag=f"st_{name}")
            nc.vector.reciprocal(out=_ap(stD[:, 0, 0], 0, [[2, cbo]]),
                                 in_=sd)
            nc.vector.tensor_mul(out=_ap(stD[:, 0, 0], 1, [[2, cbo]]),
                                 in0=mview,
                                 in1=_ap(stD[:, 0, 0], 0, [[2, cbo]]))
            nc.vector.tensor_scalar(out=_ap(stD[:, 0, 0], 1, [[2, cbo]]),
                                    in0=_ap(stD[:, 0, 0], 1, [[2, cbo]]),
                                    scalar1=-1.0, scalar2=None, op0=ALU.mult)
            outs = [stD[:, m, :] for m in range(cbo)]
            st_tiles[name] = outs
            return outs

        # ================= L1: 7x7 conv, 3 -> 64 (K=96) =====================
        _sc = nc.enter_named_scope("L1", False)[0]
        w1t = wsm.tile([96, 2, 128], BF16, name="w1t")
        nc.sync.dma_start(out=w1t, in_=w1[:, :, :])
        stt1 = stp.tile([P, 1, 144, 6], F32, name="stt1", tag="stats")
        NS1 = 36
        slabs1 = [None] * NS1

        def l1_load(s):
            sl = sb.tile([96, 8, 518], F8, name="sl1", tag="inslab")
            nc.sync.dma_start(out=sl, in_=_ap(xrep[0:96, 0, 0], s * 8 * 518,
                                              [[518, 8], [1, 518]]))
            slabs1[s] = sl

        def l1_compute(s):
            sl = slabs1[s]
            osb = osl.tile([P, 4, 512], F8, name="os1", tag="outslab")
            for k in range(4):
                pt = ps.tile([P, 512], F32, name="pt1", tag="mm")
                for d in range(2):
                    rhs = _ap(sl[:, 0, 0], 2 * k * 518 + 4 * d, [[1, 512]])
                    nc.tensor.matmul(pt, w1t[:, d, :], rhs,
                                     start=(d == 0), stop=(d == 1))
                nc.scalar.activation(out=osb[:, k, :], in_=pt, func=AF.Copy)
                nc.vector.bn_stats(out=stt1[:, 0, s * 4 + k, :],
                                   in_=osb[:, k, :])
            nc.sync.dma_start(out=_ap(h0[0:128, 0, 0], s * 4 * 512,
                                      [[512, 4], [1, 512]]),
                              in_=osb)

        for s in range(NS1 + 2):
            if s < NS1:
                l1_load(s)
            if s >= 2:
                l1_compute(s - 2)
        if debug:
            nc.sync.dma_start(out=dbg["stt1"][:, :],
                              in_=stt1.rearrange("p a b c -> p (a b c)"))

        # ================= d0: 3x3 s2, 64 -> 128 (pair-packed) =============
        nc.leave_named_scope("L1", _sc, False)
        _sc = nc.enter_named_scope("down", False)[0]
        wd0at = wsm.tile([128, 3, 128], BF16, name="wd0at")
        nc.sync.dma_start(out=wd0at, in_=wd0a[:, :, :])
        wd0bt = wsm.tile([128, 3, 128], BF16, name="wd0bt")
        nc.sync.dma_start(out=wd0bt, in_=wd0b[:, :, :])
        stt0 = stp.tile([P, 1, 72, 6], F32, name="stt0", tag="stats")
        ND0 = 36
        slabs0 = [None] * ND0

        def d0_load(s):
            y0 = s * 4
            sl = sb.tile([128, 5, 512], F8, name="sl0", tag="inslab")
            p_lo = max(y0 - 1, 0)
            nc.sync.dma_start(
                out=sl[:, p_lo - (y0 - 1):5, :],
                in_=_ap(h0[0:128, 0, 0], p_lo * 512,
                        [[512, 5 - (p_lo - (y0 - 1))], [1, 512]]))
            slabs0[s] = sl

        def d0_relu(s):
            sl = slabs0[s]
            st_ = HOLD["h0"][0]
            y0 = s * 4
            lo = 1 if y0 == 0 else 0
            cuts = [(lo, 3), (3, 5)] if s == 0 else [(lo, 5)]
            for (rl, rh) in cuts:
                nc.scalar.activation(out=sl[:, rl:rh, :],
                                     in_=sl[:, rl:rh, :],
                                     func=AF.Relu, bias=st_[:, 1:2],
                                     scale=st_[:, 0:1])
            if y0 == 0:
                nc.vector.memset(sl[:, 0:1, :], 0.0)

        def d0_compute(s):
            sl = slabs0[s]
            y0 = s * 4
            for ch in range(2):
                pt = ps.tile([P, 2, 256], F32, name="pt0", tag="mm")
                yb = 2 * ch  # local out row in strip
                first = True
                for dx in (1, 0, 2):
                    # K128 taps (pairs y), PE tile position (0, 0)
                    if dx == 0:
                        o = _ap(pt[:, 0, 0], 1, [[256, 2], [1, 255]])
                        rhs = _ap(sl[:, 0, 0], (yb + 1) * 512 + 1,
                                  [[512, 2], [2, 255]])
                    else:
                        o = pt
                        rhs = _ap(sl[:, 0, 0], (yb + 1) * 512 + dx - 1,
                                  [[512, 2], [2, 256]])
                    nc.tensor.matmul(o, wd0at[:, dx, :], rhs, start=first,
                                     stop=False)
                    first = False
                for dx in (1, 0, 2):
                    # K64 taps (ky=0, pairs y-1, j=1 half), position (64, 0)
                    if dx == 0:
                        o = _ap(pt[:, 0, 0], 1, [[256, 2], [1, 255]])
                        rhs = _ap(sl[64:128, 0, 0], yb * 512 + 1,
                                  [[512, 2], [2, 255]])
                    else:
                        o = pt
                        rhs = _ap(sl[64:128, 0, 0], yb * 512 + dx - 1,
                                  [[512, 2], [2, 256]])
                    nc.tensor.matmul(o, wd0bt[64:128, dx, :], rhs,
                                     start=False, stop=(dx == 2))
                nc.vector.bn_stats(out=stt0[:, 0, (y0 + yb) // 2, :],
                                   in_=pt.rearrange("p a b -> p (a b)"))
                osb = osl.tile([P, 2, 256], F8, name="os0", tag="outslab")
                nc.scalar.activation(out=osb, in_=pt, func=AF.Copy)
                nc.sync.dma_start(
                    out=_ap(h1[0, 0:128, 0, 0], (y0 + yb) * 256,
                            [[256, 2], [1, 256]]),
                    in_=osb)


        for s in range(ND0 + 2):
            if s < ND0:
                d0_load(s)
            if s == 1:
                HOLD["h0"] = layer_stats("h0", stt1, 1, 144, fold=True)
            if s >= 2:
                d0_compute(s - 2)
            if 1 <= s < ND0 + 1:
                d0_relu(s - 1)

        # ================= generic down layers d1..d3 ======================
        def down_layer(li, name, src, dst, in_name, cbi, cbo, Sin, Wi, nr,
                       nrc, slice_fn=None, single=False, pre=None):
            Wo = Wi // 2
            Sout = Sin // 2
            nstrip = Sout // nr
            nchunk = nr // nrc
            nent = len(LAYER_ENTRIES[name])
            stt = stp.tile([P, cbo, nent, 6], F32, name=f"std{li}",
                           tag="stats")
            rows_in = 2 * nr + 1
            slabs = [None] * nstrip

            def load(s):
                y0 = s * nr
                i0 = 2 * y0 - 1
                lo = max(i0, 0)
                sl = sb.tile([P, cbi, rows_in, Wi], F8, name=f"sld{li}",
                             tag="inslabB" if single else "inslab",
                             bufs=1 if single else None)
                for cb in range(cbi):
                    nc.sync.dma_start(
                        out=sl[:, cb, lo - i0:rows_in, :],
                        in_=_ap(src[cb, 0:P, 0, 0], lo * Wi,
                                [[Wi, rows_in - (lo - i0)], [1, Wi]]))
                slabs[s] = sl

            def relu(s):
                sl = slabs[s]
                st_in = HOLD[in_name]
                y0 = s * nr
                lo = 1 if y0 == 0 else 0
                cuts = ([(lo, 2 * nrc + 2), (2 * nrc + 2, rows_in)]
                        if s == 0 else [(lo, rows_in)])
                for cb in range(cbi):
                    for (rl, rh) in cuts:
                        nc.scalar.activation(
                            out=sl[:, cb, rl:rh, :],
                            in_=sl[:, cb, rl:rh, :], func=AF.Relu,
                            bias=st_in[cb][:, 1:2], scale=st_in[cb][:, 0:1])
                if y0 == 0:
                    nc.vector.memset(sl[:, :, 0:1, :], 0.0)

            def compute(s):
                sl = slabs[s]
                y0 = s * nr
                i0 = 2 * y0 - 1
                # relu pieces of the NEXT slab, one per (m, ch) slot
                rpieces = []
                if s + 1 < nstrip:
                    st_in = HOLD[in_name]
                    half = rows_in // 2
                    for cb in range(cbi):
                        rpieces.append((cb, 0, half))
                        rpieces.append((cb, half, rows_in))

                def emit_rpiece():
                    if rpieces:
                        cb, rl, rh = rpieces.pop(0)
                        sln = slabs[s + 1]
                        nc.scalar.activation(
                            out=sln[:, cb, rl:rh, :],
                            in_=sln[:, cb, rl:rh, :], func=AF.Relu,
                            bias=st_in[cb][:, 1:2],
                            scale=st_in[cb][:, 0:1])
                for m in range(cbo):
                    wt = wpm.tile([P, cbi, 3, 3, P],
                                  BF16 if li == 1 else F8,
                                  name=f"wtd{li}", tag="wup")
                    nc.sync.dma_start(out=wt, in_=wd[li][m])
                    osb = osl.tile([P, nr, Wo], F8, name=f"osd{li}",
                                   tag="outslab")
                    for ch in range(nchunk):
                        pt = ps.tile([P, nrc, Wo], F32, name=f"ptd{li}",
                                     tag="mm")
                        first = True
                        if cbi >= 2:
                            # fp8 weights: every tap cb-paired -> pure-DR
                            # accumulation group
                            for dx in (1, 0, 2):
                                coloff = 0 if dx == 1 else 1
                                n = Wo - 1 if dx == 0 else Wo
                                o = (pt if dx != 0 else
                                     _ap(pt[:, 0, 0], 1,
                                         [[Wo, nrc], [1, n]]))
                                rowb = 2 * (y0 + ch * nrc) - 1 - i0
                                for cbp in range(cbi // 2):
                                    for dy in range(3):
                                        boff = (2 * cbp * rows_in * Wi
                                                + (rowb + dy) * Wi + coloff)
                                        rhs = _ap(sl[:, 0, 0, 0], boff,
                                                  [[rows_in * Wi, 2],
                                                   [2 * Wi, nrc], [2, n]])
                                        last = (dx == 2 and dy == 2
                                                and cbp == cbi // 2 - 1)
                                        nc.tensor.matmul(
                                            o,
                                            wt[:, 2 * cbp:2 * cbp + 2,
                                               dy, dx, :],
                                            rhs, start=first, stop=last,
                                            perf_mode=DRM)
                                        first = False
                        else:
                            for dx in (1, 0, 2):
                                for cb in range(cbi):
                                    for dy in range(3):
                                        row0 = (2 * (y0 + ch * nrc) - 1
                                                + dy - i0)
                                        boff = cb * rows_in * Wi + row0 * Wi
                                        if dx == 0:
                                            o = _ap(pt[:, 0, 0], 1,
                                                    [[Wo, nrc], [1, Wo - 1]])
                                            rhs = _ap(sl[:, 0, 0, 0],
                                                      boff + 1,
                                                      [[2 * Wi, nrc],
                                                       [2, Wo - 1]])
                                        elif dx == 2 and Wo * 2 == Wi:
                                            o = pt
                                            rhs = _ap(sl[:, 0, 0, 0],
                                                      boff + 1,
                                                      [[2 * Wi, nrc],
                                                       [2, Wo]])
                                        else:
                                            o = pt
                                            rhs = _ap(sl[:, 0, 0, 0],
                                                      boff + dx - 1,
                                                      [[2 * Wi, nrc],
                                                       [2, Wo]])
                                        last = (dx == 2 and cb == cbi - 1
                                                and dy == 2)
                                        nc.tensor.matmul(
                                            o, wt[:, cb, dy, dx, :],
                                            rhs, start=first, stop=last)
                                        first = False
                        if slice_fn is None:
                            nc.vector.bn_stats(
                                out=stt[:, m, s * nchunk + ch, :],
                                in_=pt.rearrange("p a b -> p (a b)"))
                        else:
                            for (ei, (rlo, rhi)) in enumerate(slice_fn(ch)):
                                nc.vector.bn_stats(
                                    out=stt[:, m, ch * 2 + ei, :],
                                    in_=_ap(pt[:, 0, 0], rlo * Wo,
                                            [[1, (rhi - rlo) * Wo]]))
                        nc.scalar.activation(
                            out=osb[:, ch * nrc:(ch + 1) * nrc, :], in_=pt,
                            func=AF.Copy)
                        emit_rpiece()
                    nc.sync.dma_start(
                        out=_ap(dst[m, 0:P, 0, 0], y0 * Wo,
                                [[Wo, nr], [1, Wo]]),
                        in_=osb)

            for s in range(nstrip + 2):
                if s < nstrip:
                    load(s)
                if s == min(1, nstrip - 1) and pre is not None:
                    pre()
                if s >= 2:
                    compute(s - 2)
                if s == 1:
                    relu(0)
            return stt

        stt_d1 = down_layer(1, "h2", h1, h2, "h1", 1, 2, 144, 256, 12, 4,
                            pre=lambda: HOLD.__setitem__(
                                "h1", layer_stats("h1", stt0, 1, 72)))
        stt_d2 = down_layer(2, "h3", h2, h3, "h2", 2, 4, 72, 128, 12, 4,
                            pre=lambda: HOLD.__setitem__(
                                "h2", layer_stats("h2", stt_d1, 2, 18)))
        stt_d3 = down_layer(3, "h4", h3, h4, "h3", 4, 8, 36, 64, 18, 9,
                            slice_fn=slices_d3, single=True,
                            pre=lambda: HOLD.__setitem__(
                                "h3", layer_stats("h3", stt_d2, 4, 9)))

        # ================= up layers u0..u2 ================================
        nc.leave_named_scope("down", _sc, False)
        _sc = nc.enter_named_scope("up", False)[0]

        def up_layer(li, name, src, dst, in_name, cbi, cbo, Sin, Wi, idx_map,
                     single=False, pre=None):
            Mo = 128
            Wo = 2 * Wi
            Sout = 2 * Sin
            nstrip, subs, slfn = UP_SCHED[name]
            nro = Sout // nstrip  # out rows per strip
            nent = len(LAYER_ENTRIES[name])
            stt = stp.tile([P, cbo, nent, 6], F32, name=f"stu{li}",
                           tag="stats")
            slabs = [None] * nstrip
            srows = []
            for s in range(nstrip):
                y0 = s * nro
                i_lo = max((y0 - 1) // 2, 0)
                i_hi = min((y0 + nro) // 2 + 1, Sin)
                srows.append((i_lo, i_hi))
            rows_in = max(hi - lo for lo, hi in srows) + 1

            def load(s):
                i_lo, i_hi = srows[s]
                sl = sb.tile([P, cbi, rows_in, Wi], F8, name=f"slu{li}",
                             tag="inslabB" if single else "inslab",
                             bufs=1 if single else None)
                for cb in range(cbi):
                    nc.sync.dma_start(
                        out=sl[:, cb, 0:i_hi - i_lo, :],
                        in_=_ap(src[cb, 0:P, 0, 0], i_lo * Wi,
                                [[Wi, i_hi - i_lo], [1, Wi]]))
                slabs[s] = sl

            def relu(s):
                i_lo, i_hi = srows[s]
                sl = slabs[s]
                st_in = HOLD[in_name]
                for cb in range(cbi):
                    nc.scalar.activation(
                        out=sl[:, cb, 0:i_hi - i_lo, :],
                        in_=sl[:, cb, 0:i_hi - i_lo, :], func=AF.Relu,
                        bias=st_in[cb][:, 1:2], scale=st_in[cb][:, 0:1])
                if i_hi - i_lo < rows_in:
                    nc.vector.memset(sl[:, :, i_hi - i_lo:rows_in, :], 0.0)

            def compute(s, m, wt):
                i_lo, i_hi = srows[s]
                sl = slabs[s]
                y0 = s * nro
                osb = osl.tile([Mo, nro, Wo], F8, name=f"osu{li}",
                               tag="outslab")
                for a in range(2):
                    kys = [1] if a == 0 else [0, 2]
                    for b_ in range(2):
                        kxs = [1] if b_ == 0 else [2, 0]
                        k0 = 0
                        for su, rsub in enumerate(subs):
                            yb = y0 + a + 2 * k0
                            pt = ps.tile([Mo, rsub, Wi], F32, name=f"ptu{li}",
                                         tag="mm")
                            first = True
                            for kx in kxs:
                                j0 = (b_ + 1 - kx) // 2
                                trim = 1 if (kx == 0 and j0 == 1) else 0
                                n = Wi - 1 if trim else Wi
                                o = pt[:, :, 0:n] if trim else pt
                                for ky in kys:
                                    i_first = (yb + 1 - ky) // 2
                                    for cbp in range(cbi // 2):
                                        boff = (2 * cbp * rows_in * Wi
                                                + (i_first - i_lo) * Wi + j0)
                                        rhs = _ap(sl[:, 0, 0, 0], boff,
                                                  [[rows_in * Wi, 2],
                                                   [Wi, rsub], [1, n]])
                                        last = (kx == kxs[-1] and ky == kys[-1]
                                                and cbp == cbi // 2 - 1)
                                        nc.tensor.matmul(
                                            o,
                                            wt[:, 2 * cbp:2 * cbp + 2,
                                               ky, kx, :],
                                            rhs, start=first, stop=last,
                                            perf_mode=DRM)
                                        first = False
                            for (ei, (klo, khi)) in enumerate(slfn(s, su)):
                                nc.vector.bn_stats(
                                    out=stt[:, m, idx_map[(s, a, b_, su, ei)], :],
                                    in_=_ap(pt[:, 0, 0], klo * Wi,
                                            [[1, (khi - klo) * Wi]]))
                            oap = _ap(osb[:, 0, 0],
                                      (a + 2 * k0) * Wo + b_,
                                      [[2 * Wo, rsub], [2, Wi]])
                            nc.scalar.activation(out=oap, in_=pt,
                                                 func=AF.Copy)
                            k0 += rsub
                nc.sync.dma_start(
                    out=_ap(dst[m, 0:Mo, 0, 0], y0 * Wo, [[Wo, nro], [1, Wo]]),
                    in_=osb)

            for s in range(nstrip):
                load(s)
                if s == 0 and pre is not None:
                    pre()
                relu(s)
                for m in range(cbo):
                    wt = wpm.tile([P, cbi, 3, 3, Mo], F8, name=f"wtu{li}",
                                  tag="wup")
                    nc.sync.dma_start(out=wt, in_=wu[li][m])
                    compute(s, m, wt)
            return stt

        stt_u0 = up_layer(0, "g0", h4, g0, "h4", 8, 4, 18, 32, IDX_G0,
                          single=True,
                          pre=lambda: HOLD.__setitem__(
                              "h4", layer_stats("h4", stt_d3, 8, 4)))
        stt_u1 = up_layer(1, "g1", g0, g1, "g0", 4, 2, 36, 64, IDX_G1,
                          pre=lambda: HOLD.__setitem__(
                              "g0", layer_stats("g0", stt_u0, 4, 16)))
        stt_u2 = up_layer(2, "g2", g1, g2, "g1", 2, 1, 72, 128, IDX_G2,
                          pre=lambda: HOLD.__setitem__(
                              "g1", layer_stats("g1", stt_u1, 2, 32)))

        # ================= u3: 128 -> 64, M-packed into g3 =================
        wu3at = wsm.tile([128, 3, 128], BF16, name="wu3at")
        nc.sync.dma_start(out=wu3at, in_=wu3a[:, :, :])
        wu3bt = wsm.tile([128, 3, 64], BF16, name="wu3bt")
        nc.sync.dma_start(out=wu3bt, in_=wu3b[:, :, :])
        stt3 = stp.tile([P, 1, 144, 6], F32, name="stt3", tag="stats")
        NU3 = 6
        slabs3 = [None] * NU3

        def u3_load(s):
            q0 = s * 24
            i_hi = min(q0 + 25, 144)
            sl = sb.tile([P, 26, 256], F8, name="sl3", tag="inslab")
            nc.sync.dma_start(out=sl[:, 0:i_hi - q0, :],
                              in_=_ap(g2[0, 0:P, 0, 0], q0 * 256,
                                      [[256, i_hi - q0], [1, 256]]))
            slabs3[s] = (sl, i_hi - q0)

        def u3_relu(s):
            sl, nreal = slabs3[s]
            st_ = HOLD["g2"][0]
            cuts = [(0, 13), (13, nreal)] if s == 0 else [(0, nreal)]
            for (rl, rh) in cuts:
                nc.scalar.activation(out=sl[:, rl:rh, :],
                                     in_=sl[:, rl:rh, :],
                                     func=AF.Relu, bias=st_[:, 1:2],
                                     scale=st_[:, 0:1])
            if nreal < 26:
                nc.vector.memset(sl[:, nreal:26, :], 0.0)

        def u3_compute(s):
            sl, _n = slabs3[s]
            q0s = s * 24
            for blk in range(12):
                q0 = blk * 2  # local to slab
                pb0 = ps.tile([P, 2, 256], F32, name="pb0", tag="mm")
                pb1 = ps.tile([P, 2, 256], F32, name="pb1", tag="mm")
                # T1: full, start
                nc.tensor.matmul(pb0, wu3at[:, 0, :],
                                 _ap(sl[:, 0, 0], q0 * 256,
                                     [[256, 2], [1, 256]]),
                                 start=True, stop=False)
                # T2: rhs i=q+1, M 64..127
                nc.tensor.matmul(pb0[64:128, :, :], wu3bt[:, 0, :],
                                 _ap(sl[:, 0, 0], (q0 + 1) * 256,
                                     [[256, 2], [1, 256]]),
                                 start=False, stop=True)
                # T3: full, start
                nc.tensor.matmul(pb1, wu3at[:, 1, :],
                                 _ap(sl[:, 0, 0], q0 * 256,
                                     [[256, 2], [1, 256]]),
                                 start=True, stop=False)
                # T4: cols p+1, trim last
                nc.tensor.matmul(pb1[:, :, 0:255], wu3at[:, 2, :],
                                 _ap(sl[:, 0, 0], q0 * 256 + 1,
                                     [[256, 2], [1, 255]]),
                                 start=False, stop=False)
                # T5: rhs i=q+1 col p, M 64..127
                nc.tensor.matmul(pb1[64:128, :, :], wu3bt[:, 1, :],
                                 _ap(sl[:, 0, 0], (q0 + 1) * 256,
                                     [[256, 2], [1, 256]]),
                                 start=False, stop=False)
                # T6: rhs i=q+1 col p+1, M 64..127, trim
                nc.tensor.matmul(pb1[64:128, :, 0:255], wu3bt[:, 2, :],
                                 _ap(sl[:, 0, 0], (q0 + 1) * 256 + 1,
                                     [[256, 2], [1, 255]]),
                                 start=False, stop=True)
                eidx = (s * 12 + blk) * 2
                osb = osl.tile([P, 2, 518], F8, name="os3", tag="outslab")
                nc.scalar.activation(
                    out=_ap(osb[:, 0, 0], 3, [[518, 2], [2, 256]]), in_=pb0,
                    func=AF.Copy)
                nc.vector.tensor_copy(
                    out=_ap(osb[:, 0, 0], 4, [[518, 2], [2, 256]]), in_=pb1)
                nc.vector.bn_stats(out=stt3[:, 0, eidx, :],
                                   in_=pb0.rearrange("p a b -> p (a b)"))
                nc.vector.bn_stats(out=stt3[:, 0, eidx + 1, :],
                                   in_=pb1.rearrange("p a b -> p (a b)"))
                # g3 pair = q + 2 (pad4 top)
                nc.sync.dma_start(
                    out=_ap(g3[0:P, 0, 0], (q0s + q0 + 2) * 518,
                            [[518, 2], [1, 518]]),
                    in_=osb)
        for s in range(NU3 + 2):
            if s < NU3:
                u3_load(s)
            if s == 1:
                HOLD["g2"] = layer_stats("g2", stt_u2, 1, 84)
            if s >= 2:
                u3_compute(s - 2)
            if 1 <= s < NU3 + 1:
                u3_relu(s - 1)

        # --- g3 gutters: rows (reflect, partition-sliced) + cols ----------
        # padded row 1 <- 7, 2 <- 6, 3 <- 5 ; 292 <- 290, 293 <- 289, 294<-288
        for (d_, s_) in ((0, 8), (1, 7), (2, 6), (3, 5), (292, 290),
                         (293, 289), (294, 288), (295, 287)):
            dp, dj = divmod(d_, 2)
            sp, sj = divmod(s_, 2)
            nc.sync.dma_start(
                out=_ap(g3[dj * 64:(dj + 1) * 64, 0, 0], dp * 518, [[1, 518]]),
                in_=_ap(g3[sj * 64:(sj + 1) * 64, 0, 0], sp * 518, [[1, 518]]))
        # ================= final 7x7 + tanh + masked sum ===================
        nc.leave_named_scope("up", _sc, False)
        _sc = nc.enter_named_scope("final", False)[0]
        wfAt = wsm.tile([128, 7, 126], BF16, name="wfAt")
        nc.sync.dma_start(out=wfAt, in_=wfA[:, :, :])
        wfSt = wsm.tile([126, 7, 18], BF16, name="wfSt")
        nc.sync.dma_start(out=wfSt, in_=wfS[:, :, :])
        bft = nrm.tile([18, 1], F32, name="bft")
        nc.sync.dma_start(out=bft, in_=bfv[:, :])
        acc = nrm.tile([18, 96], F32, name="acc")
        nc.vector.memset(acc, 0.0)
        NF = 24  # strips of 12 output rows (2 y0-groups of 6)
        fslabs = [None] * NF

        def f_load(s):
            sl = sb.tile([128, 10, 518], F8, name="slF", tag="inslab")
            nc.sync.dma_start(out=sl, in_=_ap(g3[0:128, 0, 0], 6 * s * 518,
                                              [[518, 10], [1, 518]]))
            # reflect column gutters in SBUF (g3 cols 0..2/515..517 are junk)
            for k in range(3):
                nc.vector.tensor_copy(out=sl[:, :, k:k + 1],
                                      in_=sl[:, :, 6 - k:7 - k])
                nc.vector.tensor_copy(out=sl[:, :, 515 + k:516 + k],
                                      in_=sl[:, :, 513 - k:514 - k])
            mt = sb.tile([18, 4, 256], F32, name="mt", tag="mslab")
            nc.sync.dma_start(out=mt, in_=_dap(maskrep, s * 1024,
                                               [[48 * 512, 18], [1, 1024]]))
            fslabs[s] = (sl, mt)

        def f_relu(s):
            sl, _m = fslabs[s]
            st_ = HOLD["g3"][0]
            cuts = [(0, 7), (7, 10)] if s == 0 else [(0, 10)]
            for (rl, rh) in cuts:
                nc.scalar.activation(out=sl[:, rl:rh, :], in_=sl[:, rl:rh, :],
                                     func=AF.Relu,
                                     bias=st_[:, 1:2], scale=st_[:, 0:1])

        def f_compute(s):
            sl, mt = fslabs[s]
            for g in range(2):
                for hx in range(2):
                    ptA_f = psf.tile([128, 262], F32, name="ptA", tag="fa")
                    ptA = ptA_f[0:126, :]
                    for t in range(7):
                        rhs = _ap(sl[:, 0, 0], (3 * g + t) * 518 + hx * 256,
                                  [[1, 262]])
                        nc.tensor.matmul(ptA, wfAt[:, t, :], rhs,
                                         start=(t == 0), stop=(t == 6))
                    stg = osl.tile([126, 262], F8, name="stg", tag="fstg")
                    nc.vector.tensor_copy(out=stg, in_=ptA)
                    ptB_f = psf.tile([128, 256], F32, name="ptB", tag="fb")
                    ptB = ptB_f[0:18, :]
                    for dx in range(7):
                        nc.tensor.matmul(ptB, wfSt[:, dx, :],
                                         stg[:, dx:dx + 256],
                                         start=(dx == 0), stop=(dx == 6))
                    t1 = osl.tile([18, 256], F32, name="ft1", tag="ft1")
                    nc.vector.tensor_scalar(out=t1, in0=ptB, scalar1=bft,
                                            scalar2=None, op0=ALU.add)
                    nc.vector.tensor_mul(out=t1, in0=t1,
                                         in1=mt[:, 2 * g + hx, :])
                    th = osl.tile([18, 256], F32, name="fth", tag="fth")
                    ai = 4 * s + 2 * g + hx
                    nc.scalar.activation(out=th, in_=t1, func=AF.Tanh,
                                         accum_out=acc[:, ai:ai + 1])

        for s in range(NF + 2):
            if s < NF:
                f_load(s)
            if s == 1:
                HOLD["g3"] = layer_stats("g3", stt3, 1, 144, fold=True)
            if s >= 2:
                f_compute(s - 2)
            if 1 <= s < NF + 1:
                f_relu(s - 1)

        osum_t = nrm.tile([18, 1], F32, name="osum_t")
        nc.vector.tensor_reduce(out=osum_t, in_=acc, op=ALU.add,
                                axis=mybir.AxisListType.X)
        nc.sync.dma_start(out=osum[:, :], in_=osum_t)

        if debug:
            for nm, tens in [("h0", h0), ("g3", g3)]:
                sh = dbg[nm].shape
                nc.sync.dma_start(
                    out=_dap(dbg[nm], 0, [[sh[1], sh[0]], [1, sh[1]]]),
                    in_=_dap(tens, 0, [[sh[1], sh[0]], [1, sh[1]]]))
            for nm, tens, cbo, sz in [("h1", h1, 1, 144 * 256),
                                      ("h2", h2, 2, 72 * 128),
                                      ("h3", h3, 4, 36 * 64),
                                      ("h4", h4, 8, 18 * 32),
                                      ("g0", g0, 4, 36 * 64),
                                      ("g1", g1, 2, 72 * 128),
                                      ("g2", g2, 1, 144 * 256)]:
                for m in range(cbo):
                    nc.sync.dma_start(
                        out=_dap(dbg[nm], m * P * sz, [[sz, P], [1, sz]]),
                        in_=_dap(tens, m * P * sz, [[sz, P], [1, sz]]))
            for i, nm in enumerate(LAYER_ORDER):
                nc.sync.dma_start(out=dbg["st"][:, 2 * i:2 * i + 2],
                                  in_=HOLD[nm][0][:, :])
            off = 0
            for k in ["h0", "h1", "h2", "h3", "h4", "g0", "g1", "g2", "g3"]:
                n_ = CT[k]
                nc.sync.dma_start(out=dbg["sr"][off:off + n_, :],
                                  in_=stat_r[k][0:n_, :])
                nc.sync.dma_start(out=dbg["sl"][off:off + n_, :],
                                  in_=stat_l[k][:, :])
                off += n_

    nc.finalize()
    return nc


_CACHE = {}


def make_in_maps(inputs):
    wblobs = prep_weights(inputs)
    x = np.asarray(inputs["x"], np.float32)
    inst = np.asarray(inputs["inst"])
    return [prep_core_inputs(x[c // 2], inst[c // 2, 0], wblobs, c % 2)
            for c in range(8)]


def combine_outputs(res, inst):
    mask = (np.asarray(inst) == 1).astype(np.float32)  # [B,1,H,W]
    cnt = mask.sum((2, 3))  # [B,1]
    out = np.zeros((B, 3, H, W), np.float32)
    for b in range(B):
        s_top = res.results[2 * b]["osum"].reshape(6, 3).sum(0)
        s_bot = res.results[2 * b + 1]["osum"].reshape(6, 3).sum(0)
        mean = (s_top + s_bot) / cnt[b, 0]
        out[b] = mean[:, None, None] * mask[b, 0]
    return out


def kernel(**inputs):
    if "nc" not in _CACHE:
        _CACHE["nc"] = build_kernel()
    res = run_bass_kernel_spmd(_CACHE["nc"], make_in_maps(inputs),
                               core_ids=list(range(8)))
    return combine_outputs(res, inputs["inst"])



# revision 79
# speedup vs baseline: 1.0051x; 1.0051x over previous
# Trainium2 Bass kernel for nn_Encoder_81509889343552 — spatially sharded v2.
#
# Each image (B=4) is split top/bottom across a core pair (8 cores total).
# Uniform SPMD spans via exact doubling: per-core local row spans
#   h0=288, h1=144, h2=72, h3=36, h4=18, g0=36, g1=72, g2=144, g3=288, hf=288.
# InstanceNorm stats are exact: per-strip bn_stats entries are weighted by a
# host 0/1 ownership vector (each core owns its half of every layer), then a
# tiny per-layer AllReduce of (mean, var+mean^2) across the pair combines the
# halves. Junk rows near the interior boundary (from zero-pad instead of real
# neighbor data) are excluded from ownership by construction.
#
# bf16 activations/weights (f32 stats + PSUM), K-packed L1 (K=96), pair-packed
# h0 (d0 gets K=128+K=64 taps), M-packed u3 (row pairs in PE columns), fused
# masked segment mean (no hf DRAM roundtrip; tanh(mask*(x+b)) = mask*tanh(x+b)
# for a binary mask). Host combines the pair partial sums and scatters.
import sys

sys.path.insert(0, "/opt/trn_rl_repo")

import contextlib

import numpy as np
import ml_dtypes

import concourse.bass as bass
import concourse.bacc as bacc
import concourse.tile as tile
from concourse import mybir
from concourse.bass_utils import run_bass_kernel_spmd

F32 = mybir.dt.float32
F32R = mybir.dt.float32r
BF16 = mybir.dt.bfloat16
AF = mybir.ActivationFunctionType
ALU = mybir.AluOpType
BFH = ml_dtypes.bfloat16
F8 = mybir.dt.float8e4
F8H = ml_dtypes.float8_e4m3
DRM = mybir.MatmulPerfMode.DoubleRow

B, H, W = 4, 512, 512
EPS = 1e-5
P = 128
RG = [[0, 1], [2, 3], [4, 5], [6, 7]]

# local span / bottom base / owned-cutoff table (top cut = owned end,
# bot cut = owned start, in local rows)
SPAN = {
    "h0": (288, 224, 256, 32), "h1": (144, 112, 128, 16),
    "h2": (72, 56, 64, 8),     "h3": (36, 28, 32, 4),
    "h4": (18, 14, 16, 2),     "g0": (36, 28, 32, 4),
    "g1": (72, 56, 64, 8),     "g2": (144, 112, 128, 16),
    "g3": (288, 224, 256, 32),
}


def _ap(base, extra_off, dims):
    return bass.AP(
        tensor=base.tensor,
        offset=base.offset + extra_off,
        ap=[list(base.ap[0])] + [list(d) for d in dims],
    )


def _dap(handle, off, dims):
    return bass.AP(tensor=handle, offset=off, ap=[list(d) for d in dims])


# ---------------------------------------------------------------------------
# Stats-entry enumeration (shared between host weight gen and device build).
# Each entry is (row_lo, row_hi) half-open in local layer rows; the device
# emits bn_stats in exactly this order per channel-block.
# ---------------------------------------------------------------------------

def entries_L1():
    return [(2 * k, 2 * k + 2) for k in range(144)]


def entries_d0():
    return [(2 * e, 2 * e + 2) for e in range(72)]


def entries_down(nstrip, nchunk, nrc):
    out = []
    for s in range(nstrip):
        for c in range(nchunk):
            lo = s * nchunk * nrc + c * nrc
            out.append((lo, lo + nrc))
    return out


def slices_d3(chunk):
    # chunk 0 = rows 0..8, chunk 1 = rows 9..17; cutoffs at rows {2, 16}
    return [(0, 2), (2, 9)] if chunk == 0 else [(0, 7), (7, 9)]


def entries_d3():
    return [(0, 2), (2, 9), (9, 16), (16, 18)]


# --- up-layer su/slice schedules -------------------------------------------
# UP_SCHED[name] = (nstrip, subs_per_strip, slices(s, su) -> [(klo,khi)])
def _u0_slices(s, su):
    return [(0, 2), (2, 9)] if su == 0 else [(0, 7), (7, 9)]


def _u1_slices(s, su):
    if s == 0:
        return [(0, 4), (4, 8)] if su == 0 else \
            ([(0, 8)] if su == 1 else [(0, 2)])
    return [(0, 6), (6, 8)] if su == 1 else \
        ([(0, 8)] if su == 0 else [(0, 2)])


def _u2_slices(s, su):
    if s == 3 and su == 2:
        return [(0, 2), (2, 4)]
    return [(0, 4)] if su < 4 else [(0, 2)]


UP_SCHED = {
    "g0": (1, [9, 9], _u0_slices),
    "g1": (2, [8, 8, 2], _u1_slices),
    "g2": (4, [4, 4, 4, 4, 2], _u2_slices),
}


def up_entries_and_index(name, Sout):
    nstrip, subs, slfn = UP_SCHED[name]
    nro = Sout // nstrip
    ents, idx = [], {}
    for s in range(nstrip):
        y0 = s * nro
        for a in range(2):
            for b_ in range(2):
                for su in range(len(subs)):
                    k0 = sum(subs[:su])
                    for ei, (klo, khi) in enumerate(slfn(s, su)):
                        idx[(s, a, b_, su, ei)] = len(ents)
                        ents.append((y0 + a + 2 * (k0 + klo),
                                     y0 + a + 2 * (k0 + khi - 1) + 1))
    return ents, idx


ENT_G0, IDX_G0 = up_entries_and_index("g0", 36)
ENT_G1, IDX_G1 = up_entries_and_index("g1", 72)
ENT_G2, IDX_G2 = up_entries_and_index("g2", 144)


def entries_u3():
    # 6 strips x 12 blocks x 2 (pb0, pb1); block = 2 out-row pairs = 4 rows
    out = []
    for s in range(6):
        for blk in range(12):
            q0 = s * 24 + blk * 2
            out.append((2 * q0, 2 * q0 + 4))
            out.append((2 * q0, 2 * q0 + 4))
    return out


LAYER_ENTRIES = {
    "h0": entries_L1(), "h1": entries_d0(),
    "h2": entries_down(6, 3, 4), "h3": entries_down(3, 3, 4),
    "h4": entries_d3(), "g0": ENT_G0, "g1": ENT_G1,
    "g2": ENT_G2, "g3": entries_u3(),
}
LAYER_ORDER = ["h0", "h1", "h2", "h3", "h4", "g0", "g1", "g2", "g3"]
LAYER_CBO = {"h0": 1, "h1": 1, "h2": 2, "h3": 4, "h4": 8,
             "g0": 4, "g1": 2, "g2": 1, "g3": 1}


def statw_vector(half):
    """Concatenated per-entry 0/1 weights (x6 fields, replicated per
    channel-block) for this core half."""
    vals = []
    offs = {}
    for name in LAYER_ORDER:
        S, bbase, cut_top, cut_bot = SPAN[name]
        offs[name] = len(vals)
        lw = []
        for (lo, hi) in LAYER_ENTRIES[name]:
            if half == 0:
                w = 1.0 if hi <= cut_top else 0.0
                assert hi <= cut_top or lo >= cut_top, (name, lo, hi)
            else:
                w = 1.0 if lo >= cut_bot else 0.0
                assert lo >= cut_bot or hi <= cut_bot, (name, lo, hi)
            lw.extend([w] * 6)
        vals.extend(lw * LAYER_CBO[name])
    return np.asarray(vals, np.float32), offs


STATW_TOP, STATW_OFFS = statw_vector(0)
STATW_BOT, _ = statw_vector(1)
NSTATW = len(STATW_TOP)


# ---------------------------------------------------------------------------
# Host-side weight preprocessing (all lhsT blobs in SBUF layout, bf16)
# ---------------------------------------------------------------------------

def prep_weights(inp):
    w = {}
    # L1: K = (ci, dy, l4) = 96; M = r*64+co; matmul d in {0,1}: kx = 4d + l
    w0 = np.asarray(inp["w0"], np.float32)  # [64, 3, 7, 7]
    w1 = np.zeros((96, 2, 128), np.float32)
    for ci in range(3):
        for dy in range(8):
            for l in range(4):
                p = ci * 32 + dy * 4 + l
                for d in range(2):
                    kx = 4 * d + l
                    if kx > 6:
                        continue
                    for r in range(2):
                        ky = dy - r
                        if 0 <= ky <= 6:
                            w1[p, d, r * 64:(r + 1) * 64] = w0[:, ci, ky, kx]
    w["w1"] = w1.astype(BFH)

    # d0 (pair-packed h0): K128 tap (pair y): rows (j,c): ky=1+j;
    # K64 tap (pair y-1, j=1 partitions 64..127): ky=0.
    dw0 = np.asarray(inp["dw0"], np.float32)  # [128, 64, 3, 3]
    wd0a = np.zeros((128, 3, 128), np.float32)
    wd0b = np.zeros((128, 3, 128), np.float32)
    for dx in range(3):
        for j in range(2):
            for c in range(64):
                wd0a[j * 64 + c, dx, :] = dw0[:, c, 1 + j, dx]
        for c in range(64):
            wd0b[64 + c, dx, :] = dw0[:, c, 0, dx]
    w["wd0a"] = wd0a.astype(BFH)
    w["wd0b"] = wd0b.astype(BFH)

    # d1..d3: [cbo, K=128, cbi, 3, 3, M=128] (k-major per m-block)
    c = 128
    for i in (1, 2, 3):
        dw = np.asarray(inp[f"dw{i}"], np.float32)  # [2c, c, 3, 3]
        cbo, cbi = (2 * c) // P, c // P
        blob = np.zeros((cbo, P, cbi, 3, 3, P), np.float32)
        for m in range(cbo):
            for cb in range(cbi):
                for dy in range(3):
                    for dx in range(3):
                        blob[m, :, cb, dy, dx, :] = \
                            dw[m * P:(m + 1) * P, cb * P:(cb + 1) * P, dy, dx].T
        w[f"wd{i}"] = blob.astype(BFH if i == 1 else F8H)
        c *= 2

    # u0..u2 (torch convT layout uw [Cin, Cout, 3, 3]):
    # [cbo, K=128, cbi, 3, 3, Mo]
    for i in (0, 1, 2):
        uw = np.asarray(inp[f"uw{i}"], np.float32)
        Cin_, Cout_ = uw.shape[0], uw.shape[1]
        cbi, cbo, Mo = Cin_ // P, max(Cout_ // P, 1), min(Cout_, P)
        blob = np.zeros((cbo, P, cbi, 3, 3, Mo), np.float32)
        for m in range(cbo):
            for cb in range(cbi):
                for ky in range(3):
                    for kx in range(3):
                        blob[m, :, cb, ky, kx, :] = \
                            uw[cb * P:(cb + 1) * P, m * Mo:(m + 1) * Mo, ky, kx]
        w[f"wu{i}"] = blob.astype(F8H)

    # u3 M-packed: psum partition q = j*64 + c (j = out row parity).
    # T1 (b0, rhs i=q,  col p):  j0: (ky1,kx1); j1: (ky2,kx1)
    # T2 (b0, rhs i=q+1,col p):  j1: (ky0,kx1)   [M 64..127]
    # T3 (b1, rhs i=q,  col p):  j0: (ky1,kx2); j1: (ky2,kx2)
    # T4 (b1, rhs i=q,  col p+1):j0: (ky1,kx0); j1: (ky2,kx0)
    # T5 (b1, rhs i=q+1,col p):  j1: (ky0,kx2)
    # T6 (b1, rhs i=q+1,col p+1):j1: (ky0,kx0)
    uw3 = np.asarray(inp["uw3"], np.float32)  # [128, 64, 3, 3]
    wa = np.zeros((128, 3, 128), np.float32)  # T1, T3, T4
    wb = np.zeros((128, 3, 64), np.float32)   # T2, T5, T6
    for t, (ky0_, kx0_, ky1_, kx1_) in enumerate(
            [(1, 1, 2, 1), (1, 2, 2, 2), (1, 0, 2, 0)]):
        wa[:, t, 0:64] = uw3[:, :, ky0_, kx0_]
        wa[:, t, 64:128] = uw3[:, :, ky1_, kx1_]
    for t, (ky_, kx_) in enumerate([(0, 1), (0, 2), (0, 0)]):
        wb[:, t, :] = uw3[:, :, ky_, kx_]
    w["wu3a"] = wa.astype(BFH)
    w["wu3b"] = wb.astype(BFH)

    # final conv stage A: K = (j, c) over 4 g3-pairs; slab pair t holds padded
    # rows 6k+2t+j for strip y0=6k; out row y0+r reads padded y0+1+ky'
    # (pad offset 4 at top => padded row == local row + 4... see g3 layout):
    # tap ky' = 2t + j - r - 1; M = r*21 + dx*3 + co.
    wf = np.asarray(inp["wf"], np.float32)  # [3, 64, 7, 7]
    wfA = np.zeros((128, 7, 126), np.float32)
    for t in range(7):
        for j in range(2):
            for r in range(6):
                ky = 2 * t + j - r - 1
                if 0 <= ky <= 6:
                    for dx in range(7):
                        for co in range(3):
                            wfA[j * 64:(j + 1) * 64, t, r * 21 + dx * 3 + co] = \
                                wf[co, :, ky, dx]
    w["wfA"] = wfA.astype(BFH)
    wfS = np.zeros((126, 7, 18), np.float32)
    for dx in range(7):
        for r in range(6):
            for co in range(3):
                wfS[r * 21 + dx * 3 + co, dx, r * 3 + co] = 1.0
    w["wfS"] = wfS.astype(BFH)
    bf = np.asarray(inp["bf"], np.float32)
    w["bfv"] = np.tile(bf, 6).reshape(18, 1).astype(np.float32)

    # j-fold (average partitions c and c+64) for h0 / g3 stats
    wfold = np.zeros((128, 64), np.float32)
    for j in range(2):
        for c_ in range(64):
            wfold[j * 64 + c_, c_] = 0.5
    w["wfold"] = wfold
    return w


def prep_core_inputs(x_img, inst_img, wblobs, half):
    """Per-core inputs: xrep (K-packed padded x slice), maskrep, statw."""
    xpad = np.pad(np.asarray(x_img, np.float32), ((0, 0), (3, 3), (3, 3)),
                  mode="reflect")  # [3, 518, 518]
    r0 = 0 if half == 0 else 224
    xrep = np.zeros((96, 288, 518), np.float32)
    for ci in range(3):
        for dy in range(8):
            for l in range(4):
                p = ci * 32 + dy * 4 + l
                hi = min(r0 + dy + 288, 518)
                rows = xpad[ci, r0 + dy:hi, :]
                xrep[p, :hi - (r0 + dy), :518 - l] = rows[:, l:]
    mask = (np.asarray(inst_img) == 1).astype(np.float32)  # [512, 512]
    base = 0 if half == 0 else 224
    maskrep = np.zeros((18, 48, 2, 256), np.float32)
    for s in range(48):
        for r in range(6):
            y = 6 * s + r
            gy = base + y
            owned = (y < 256) if half == 0 else (y >= 32)
            if owned:
                row = mask[gy]
                for co in range(3):
                    maskrep[r * 3 + co, s, 0, :] = row[:256]
                    maskrep[r * 3 + co, s, 1, :] = row[256:]
    m = {
        "xrep": xrep.astype(F8H),
        "maskrep": maskrep.reshape(18, 48 * 2 * 256),
        "statw": STATW_TOP if half == 0 else STATW_BOT,
    }
    m.update(wblobs)
    return m


# ---------------------------------------------------------------------------
# Device kernel
# ---------------------------------------------------------------------------

def build_kernel(debug=False):
    nc = bacc.Bacc(None, target_bir_lowering=False, num_devices=8)

    xrep = nc.dram_tensor("xrep", [96, 288, 518], F8, kind="ExternalInput")
    maskrep = nc.dram_tensor("maskrep", [18, 48 * 2 * 256], F32,
                             kind="ExternalInput")
    statw = nc.dram_tensor("statw", [NSTATW], F32, kind="ExternalInput")
    w1 = nc.dram_tensor("w1", [96, 2, 128], BF16, kind="ExternalInput")
    wd0a = nc.dram_tensor("wd0a", [128, 3, 128], BF16, kind="ExternalInput")
    wd0b = nc.dram_tensor("wd0b", [128, 3, 128], BF16, kind="ExternalInput")
    wd = {}
    c = 128
    for i in (1, 2, 3):
        cbo, cbi = (2 * c) // P, c // P
        wd[i] = nc.dram_tensor(f"wd{i}", [cbo, P, cbi, 3, 3, P],
                               BF16 if i == 1 else F8,
                               kind="ExternalInput")
        c *= 2
    wu = {}
    c = 1024
    for i in (0, 1, 2):
        cbi, cbo, Mo = c // P, max((c // 2) // P, 1), min(c // 2, P)
        wu[i] = nc.dram_tensor(f"wu{i}", [cbo, P, cbi, 3, 3, Mo], F8,
                               kind="ExternalInput")
        c //= 2
    wu3a = nc.dram_tensor("wu3a", [128, 3, 128], BF16, kind="ExternalInput")
    wu3b = nc.dram_tensor("wu3b", [128, 3, 64], BF16, kind="ExternalInput")
    wfA = nc.dram_tensor("wfA", [128, 7, 126], BF16, kind="ExternalInput")
    wfS = nc.dram_tensor("wfS", [126, 7, 18], BF16, kind="ExternalInput")
    bfv = nc.dram_tensor("bfv", [18, 1], F32, kind="ExternalInput")
    wfold = nc.dram_tensor("wfold", [128, 64], F32, kind="ExternalInput")

    h0 = nc.dram_tensor("h0", [128, 144, 512], F8)  # pair-packed (j,c)
    h1 = nc.dram_tensor("h1", [1, 128, 144, 256], F8)
    h2 = nc.dram_tensor("h2", [2, 128, 72, 128], F8)
    h3 = nc.dram_tensor("h3", [4, 128, 36, 64], F8)
    h4 = nc.dram_tensor("h4", [8, 128, 18, 32], F8)
    g0 = nc.dram_tensor("g0", [4, 128, 36, 64], F8)
    g1 = nc.dram_tensor("g1", [2, 128, 72, 128], F8)
    g2 = nc.dram_tensor("g2", [1, 128, 144, 256], F8)
    g3 = nc.dram_tensor("g3", [128, 148, 518], F8)  # pair-packed, pad4 top
    # per-layer stats scratch in DRAM + allreduced copy
    CT = {"h0": 64, "h1": 128, "h2": 256, "h3": 512, "h4": 1024,
          "g0": 512, "g1": 256, "g2": 128, "g3": 64}
    stat_l = {k: nc.dram_tensor(f"stl_{k}", [v, 2], F32)
              for k, v in CT.items()}
    stat_r = {k: nc.dram_tensor(f"str_{k}", [2 * v, 2], F32)
              for k, v in CT.items()}
    osum = nc.dram_tensor("osum", [18, 1], F32, kind="ExternalOutput")

    dbg = {}
    if debug:
        for nm, sh in [("h0", [128, 144 * 512]), ("h1", [128, 144 * 256]),
                       ("h2", [256, 72 * 128]), ("h3", [512, 36 * 64]),
                       ("h4", [1024, 18 * 32]), ("g0", [512, 36 * 64]),
                       ("g1", [256, 72 * 128]), ("g2", [128, 144 * 256]),
                       ("g3", [128, 148 * 518])]:
            dbg[nm] = nc.dram_tensor("dbg_" + nm, sh, BF16,
                                     kind="ExternalOutput")
        dbg["st"] = nc.dram_tensor("dbg_st", [128, 2 * 9], F32,
                                   kind="ExternalOutput")
        dbg["sr"] = nc.dram_tensor("dbg_sr", [sum(CT.values()), 2], F32,
                                   kind="ExternalOutput")
        dbg["sl"] = nc.dram_tensor("dbg_sl", [sum(CT.values()), 2], F32,
                                   kind="ExternalOutput")
        dbg["stt1"] = nc.dram_tensor("dbg_stt1", [128, 144 * 6], F32,
                                     kind="ExternalOutput")

    with tile.TileContext(nc) as tc, contextlib.ExitStack() as ctx:
        sb = ctx.enter_context(tc.tile_pool(name="sb", bufs=3))
        osl = ctx.enter_context(tc.tile_pool(name="osl", bufs=2))
        wsm = ctx.enter_context(tc.tile_pool(name="wsm", bufs=1))
        wpm = ctx.enter_context(tc.tile_pool(name="wpm", bufs=2))
        nrm = ctx.enter_context(tc.tile_pool(name="nrm", bufs=1))
        stp = ctx.enter_context(tc.tile_pool(name="stp", bufs=1))
        ps = ctx.enter_context(tc.tile_pool(name="ps", bufs=4, space="PSUM"))
        psf = ctx.enter_context(tc.tile_pool(name="psf", bufs=2, space="PSUM"))

        eps_t = nrm.tile([P, 1], F32, name="eps_t")
        nc.vector.memset(eps_t, EPS)
        wfoldt = nrm.tile([P, 64], F32, name="wfoldt")
        nc.sync.dma_start(out=wfoldt, in_=wfold[:, :])

        # broadcast per-entry stat weights once: [128, NSTATW]
        # (NSTATW ~ 3.5k floats -> 14KB/partition; fine)
        wst_t = nrm.tile([P, NSTATW], F32, name="wst_t")
        nc.gpsimd.dma_start(out=wst_t,
                            in_=_dap(statw, 0, [[0, P], [1, NSTATW]]))

        st_tiles = {}
        HOLD = {}

        def layer_stats(name, stt, cbo, nent, fold=False):
            """stt [128, cbo, nent, 6] -> list of [128, 2] (scale, bias) APs
            per channel block. Weighted raw sums (NaN-proof, exact), batched
            over channel blocks; pairwise AllReduce of (mean, E)."""
            off = STATW_OFFS[name]
            n6 = cbo * nent * 6
            wl = {"h0": 512, "h1": 256, "h2": 128, "h3": 64, "h4": 32,
                  "g0": 64, "g1": 128, "g2": 256, "g3": 512}[name]
            npart = (wl // 2) * wl // (2 if fold else 1)
            tw = stp.tile([P, cbo, nent, 6], F32, name=f"tw_{name}", tag="tw")
            nc.vector.tensor_mul(
                out=tw.rearrange("p a b c -> p (a b c)"),
                in0=stt.rearrange("p a b c -> p (a b c)"),
                in1=wst_t[:, off:off + n6])
            cm = stp.tile([P, cbo, nent, 2], F32, name=f"cm_{name}", tag="cm")
            nc.vector.tensor_mul(
                out=cm.rearrange("p a b c -> p (a b c)"),
                in0=_ap(tw[:, 0, 0, 0], 0, [[6, cbo * nent], [3, 2]]),
                in1=_ap(tw[:, 0, 0, 0], 1, [[6, cbo * nent], [3, 2]]))
            e2 = stp.tile([P, cbo, nent, 2], F32, name=f"e2_{name}", tag="e2")
            nc.vector.tensor_mul(
                out=e2.rearrange("p a b c -> p (a b c)"),
                in0=cm.rearrange("p a b c -> p (a b c)"),
                in1=_ap(tw[:, 0, 0, 0], 1, [[6, cbo * nent], [3, 2]]))
            nc.vector.tensor_add(
                out=e2.rearrange("p a b c -> p (a b c)"),
                in0=e2.rearrange("p a b c -> p (a b c)"),
                in1=_ap(tw[:, 0, 0, 0], 2, [[6, cbo * nent], [3, 2]]))
            s1 = stp.tile([P, cbo], F32, name=f"s1_{name}", tag="s1")
            nc.vector.tensor_reduce(out=s1,
                                    in_=cm.rearrange("p a b c -> p a (b c)"),
                                    op=ALU.add, axis=mybir.AxisListType.X)
            s2 = stp.tile([P, cbo], F32, name=f"s2_{name}", tag="s2")
            nc.vector.tensor_reduce(out=s2,
                                    in_=e2.rearrange("p a b c -> p a (b c)"),
                                    op=ALU.add, axis=mybir.AxisListType.X)
            me = stp.tile([P, cbo, 2], F32, name=f"me_{name}", tag="me")
            nc.vector.tensor_scalar(out=me[:, :, 0:1], in0=s1,
                                    scalar1=1.0 / npart, scalar2=None,
                                    op0=ALU.mult)
            nc.vector.tensor_scalar(out=me[:, :, 1:2], in0=s2,
                                    scalar1=1.0 / npart, scalar2=None,
                                    op0=ALU.mult)
            if fold:
                pm = psf.tile([64, 2], F32, name=f"pm_{name}", tag="mini",
                              bufs=1)
                nc.tensor.matmul(pm, wfoldt, me[:, 0, :], start=True,
                                 stop=True)
                mef = stp.tile([64, 2], F32, name=f"mef_{name}", tag="mef")
                nc.scalar.activation(out=mef, in_=pm, func=AF.Copy)
                nc.gpsimd.dma_start(out=stat_l[name][0:64, :], in_=mef)
            else:
                nc.gpsimd.dma_start(
                    out=_dap(stat_l[name], 0, [[2, P], [256, cbo], [1, 2]]),
                    in_=me)
            nc.gpsimd.collective_compute(
                "AllGather", ALU.bypass, RG,
                ins=[stat_l[name][:, :]], outs=[stat_r[name][:, :]])
            # result loads go on the scalar queue so slab loads on sync are
            # not head-of-line blocked behind the collective
            ct_ = CT[name]
            lr = stp.tile([P, cbo, 2], F32, name=f"lr_{name}", tag="lr")
            lrb = stp.tile([P, cbo, 2], F32, name=f"lrb_{name}", tag="lrb")
            if fold:
                nc.scalar.dma_start(out=lr[0:64, 0, :],
                                    in_=stat_r[name][0:64, :])
                nc.sync.dma_start(out=lr[64:128, 0, :],
                                    in_=stat_r[name][0:64, :])
                nc.scalar.dma_start(out=lrb[0:64, 0, :],
                                    in_=stat_r[name][64:128, :])
                nc.gpsimd.dma_start(out=lrb[64:128, 0, :],
                                    in_=stat_r[name][64:128, :])
            else:
                nc.scalar.dma_start(
                    out=lr,
                    in_=_dap(stat_r[name], 0, [[2, P], [256, cbo], [1, 2]]))
                nc.gpsimd.dma_start(
                    out=lrb,
                    in_=_dap(stat_r[name], 2 * ct_,
                             [[2, P], [256, cbo], [1, 2]]))
            nc.vector.tensor_add(out=lr.rearrange("p a b -> p (a b)"),
                                 in0=lr.rearrange("p a b -> p (a b)"),
                                 in1=lrb.rearrange("p a b -> p (a b)"))
            t0 = stp.tile([P, cbo, 2], F32, name=f"t0_{name}", tag="t0")
            nc.vector.tensor_scalar(out=t0.rearrange("p a b -> p (a b)"),
                                    in0=lr.rearrange("p a b -> p (a b)"),
                                    scalar1=0.5, scalar2=None, op0=ALU.mult)
            mview = _ap(t0[:, 0, 0], 0, [[2, cbo]])
            eview = _ap(t0[:, 0, 0], 1, [[2, cbo]])
            var = stp.tile([P, cbo], F32, name=f"var_{name}", tag="var")
            nc.vector.tensor_mul(out=var, in0=mview, in1=mview)
            nc.vector.tensor_sub(out=var, in0=eview, in1=var)
            sd = stp.tile([P, cbo], F32, name=f"sd_{name}", tag="sd")
            nc.scalar.activation(out=sd, in_=var, func=AF.Sqrt,
                                 bias=eps_t, scale=1.0)
            stD = nrm.tile([P, cbo, 2], F32, name=f"st_{name}",
                           tag=f"st_{name}")
            nc.vector.reciprocal(out=_ap(stD[:, 0, 0], 0, [[2, cbo]]),
                                 in_=sd)
            nc.vector.tensor_mul(out=_ap(stD[:, 0, 0], 1, [[2, cbo]]),
                                 in0=mview,
                                 in1=_ap(stD[:, 0, 0], 0, [[2, cbo]]))
            nc.vector.tensor_scalar(out=_ap(stD[:, 0, 0], 1, [[2, cbo]]),
                                    in0=_ap(stD[:, 0, 0], 1, [[2, cbo]]),
                                    scalar1=-1.0, scalar2=None, op0=ALU.mult)
            outs = [stD[:, m, :] for m in range(cbo)]
            st_tiles[name] = outs
            return outs

        # ================= L1: 7x7 conv, 3 -> 64 (K=96) =====================
        _sc = nc.enter_named_scope("L1", False)[0]
        w1t = wsm.tile([96, 2, 128], BF16, name="w1t")
        nc.sync.dma_start(out=w1t, in_=w1[:, :, :])
        stt1 = stp.tile([P, 1, 144, 6], F32, name="stt1", tag="stats")
        NS1 = 36
        slabs1 = [None] * NS1

        def l1_load(s):
            sl = sb.tile([96, 8, 518], F8, name="sl1", tag="inslab")
            nc.sync.dma_start(out=sl, in_=_ap(xrep[0:96, 0, 0], s * 8 * 518,
                                              [[518, 8], [1, 518]]))
            slabs1[s] = sl

        def l1_compute(s):
            sl = slabs1[s]
            osb = osl.tile([P, 4, 512], F8, name="os1", tag="outslab")
            for k in range(4):
                pt = ps.tile([P, 512], F32, name="pt1", tag="mm")
                for d in range(2):
                    rhs = _ap(sl[:, 0, 0], 2 * k * 518 + 4 * d, [[1, 512]])
                    nc.tensor.matmul(pt, w1t[:, d, :], rhs,
                                     start=(d == 0), stop=(d == 1))
                nc.scalar.activation(out=osb[:, k, :], in_=pt, func=AF.Copy)
                nc.vector.bn_stats(out=stt1[:, 0, s * 4 + k, :],
                                   in_=osb[:, k, :])
            nc.sync.dma_start(out=_ap(h0[0:128, 0, 0], s * 4 * 512,
                                      [[512, 4], [1, 512]]),
                              in_=osb)

        for s in range(NS1 + 2):
            if s < NS1:
                l1_load(s)
            if s >= 2:
                l1_compute(s - 2)
        if debug:
            nc.sync.dma_start(out=dbg["stt1"][:, :],
                              in_=stt1.rearrange("p a b c -> p (a b c)"))

        # ================= d0: 3x3 s2, 64 -> 128 (pair-packed) =============
        nc.leave_named_scope("L1", _sc, False)
        _sc = nc.enter_named_scope("down", False)[0]
        wd0at = wsm.tile([128, 3, 128], BF16, name="wd0at")
        nc.sync.dma_start(out=wd0at, in_=wd0a[:, :, :])
        wd0bt = wsm.tile([128, 3, 128], BF16, name="wd0bt")
        nc.sync.dma_start(out=wd0bt, in_=wd0b[:, :, :])
        stt0 = stp.tile([P, 1, 72, 6], F32, name="stt0", tag="stats")
        ND0 = 36
        slabs0 = [None] * ND0

        def d0_load(s):
            y0 = s * 4
            sl = sb.tile([128, 5, 512], F8, name="sl0", tag="inslab")
            p_lo = max(y0 - 1, 0)
            nc.sync.dma_start(
                out=sl[:, p_lo - (y0 - 1):5, :],
                in_=_ap(h0[0:128, 0, 0], p_lo * 512,
                        [[512, 5 - (p_lo - (y0 - 1))], [1, 512]]))
            slabs0[s] = sl

        def d0_relu(s):
            sl = slabs0[s]
            st_ = HOLD["h0"][0]
            y0 = s * 4
            lo = 1 if y0 == 0 else 0
            cuts = [(lo, 3), (3, 5)] if s == 0 else [(lo, 5)]
            for (rl, rh) in cuts:
                nc.scalar.activation(out=sl[:, rl:rh, :],
                                     in_=sl[:, rl:rh, :],
                                     func=AF.Relu, bias=st_[:, 1:2],
                                     scale=st_[:, 0:1])
            if y0 == 0:
                nc.vector.memset(sl[:, 0:1, :], 0.0)

        def d0_compute(s):
            sl = slabs0[s]
            y0 = s * 4
            for ch in range(2):
                pt = ps.tile([P, 2, 256], F32, name="pt0", tag="mm")
                yb = 2 * ch  # local out row in strip
                first = True
                for dx in (1, 0, 2):
                    # K128 taps (pairs y), PE tile position (0, 0)
                    if dx == 0:
                        o = _ap(pt[:, 0, 0], 1, [[256, 2], [1, 255]])
                        rhs = _ap(sl[:, 0, 0], (yb + 1) * 512 + 1,
                                  [[512, 2], [2, 255]])
                    else:
                        o = pt
                        rhs = _ap(sl[:, 0, 0], (yb + 1) * 512 + dx - 1,
                                  [[512, 2], [2, 256]])
                    nc.tensor.matmul(o, wd0at[:, dx, :], rhs, start=first,
                                     stop=False)
                    first = False
                for dx in (1, 0, 2):
                    # K64 taps (ky=0, pairs y-1, j=1 half), position (64, 0)
                    if dx == 0:
                        o = _ap(pt[:, 0, 0], 1, [[256, 2], [1, 255]])
                        rhs = _ap(sl[64:128, 0, 0], yb * 512 + 1,
                                  [[512, 2], [2, 255]])
                    else:
                        o = pt
                        rhs = _ap(sl[64:128, 0, 0], yb * 512 + dx - 1,
                                  [[512, 2], [2, 256]])
                    nc.tensor.matmul(o, wd0bt[64:128, dx, :], rhs,
                                     start=False, stop=(dx == 2))
                nc.vector.bn_stats(out=stt0[:, 0, (y0 + yb) // 2, :],
                                   in_=pt.rearrange("p a b -> p (a b)"))
                osb = osl.tile([P, 2, 256], F8, name="os0", tag="outslab")
                nc.scalar.activation(out=osb, in_=pt, func=AF.Copy)
                nc.sync.dma_start(
                    out=_ap(h1[0, 0:128, 0, 0], (y0 + yb) * 256,
                            [[256, 2], [1, 256]]),
                    in_=osb)

        for s in range(ND0 + 2):
            if s < ND0:
                d0_load(s)
            if s == 1:
                HOLD["h0"] = layer_stats("h0", stt1, 1, 144, fold=True)
            if s >= 2:
                d0_compute(s - 2)
            if 1 <= s < ND0 + 1:
                d0_relu(s - 1)

        # ================= generic down layers d1..d3 ======================
        def down_layer(li, name, src, dst, in_name, cbi, cbo, Sin, Wi, nr,
                       nrc, slice_fn=None, single=False, pre=None):
            Wo = Wi // 2
            Sout = Sin // 2
            nstrip = Sout // nr
            nchunk = nr // nrc
            nent = len(LAYER_ENTRIES[name])
            stt = stp.tile([P, cbo, nent, 6], F32, name=f"std{li}",
                           tag="stats")
            rows_in = 2 * nr + 1
            slabs = [None] * nstrip

            def load(s):
                y0 = s * nr
                i0 = 2 * y0 - 1
                lo = max(i0, 0)
                sl = sb.tile([P, cbi, rows_in, Wi], F8, name=f"sld{li}",
                             tag="inslabB" if single else "inslab",
                             bufs=1 if single else None)
                for cb in range(cbi):
                    nc.sync.dma_start(
                        out=sl[:, cb, lo - i0:rows_in, :],
                        in_=_ap(src[cb, 0:P, 0, 0], lo * Wi,
                                [[Wi, rows_in - (lo - i0)], [1, Wi]]))
                slabs[s] = sl

            def relu(s):
                sl = slabs[s]
                st_in = HOLD[in_name]
                y0 = s * nr
                lo = 1 if y0 == 0 else 0
                cuts = ([(lo, 2 * nrc + 2), (2 * nrc + 2, rows_in)]
                        if s == 0 else [(lo, rows_in)])
                for cb in range(cbi):
                    for (rl, rh) in cuts:
                        nc.scalar.activation(
                            out=sl[:, cb, rl:rh, :],
                            in_=sl[:, cb, rl:rh, :], func=AF.Relu,
                            bias=st_in[cb][:, 1:2], scale=st_in[cb][:, 0:1])
                if y0 == 0:
                    nc.vector.memset(sl[:, :, 0:1, :], 0.0)

            def compute(s):
                sl = slabs[s]
                y0 = s * nr
                i0 = 2 * y0 - 1
                for m in range(cbo):
                    wt = wpm.tile([P, cbi, 3, 3, P],
                                  BF16 if li == 1 else F8,
                                  name=f"wtd{li}", tag="wup")
                    nc.sync.dma_start(out=wt, in_=wd[li][m])
                    osb = osl.tile([P, nr, Wo], F8, name=f"osd{li}",
                                   tag="outslab")
                    for ch in range(nchunk):
                        pt = ps.tile([P, nrc, Wo], F32, name=f"ptd{li}",
                                     tag="mm")
                        first = True
                        if cbi >= 2:
                            # fp8 weights: every tap cb-paired -> pure-DR
                            # accumulation group
                            for dx in (1, 0, 2):
                                coloff = 0 if dx == 1 else 1
                                n = Wo - 1 if dx == 0 else Wo
                                o = (pt if dx != 0 else
                                     _ap(pt[:, 0, 0], 1,
                                         [[Wo, nrc], [1, n]]))
                                rowb = 2 * (y0 + ch * nrc) - 1 - i0
                                for cbp in range(cbi // 2):
                                    for dy in range(3):
                                        boff = (2 * cbp * rows_in * Wi
                                                + (rowb + dy) * Wi + coloff)
                                        rhs = _ap(sl[:, 0, 0, 0], boff,
                                                  [[rows_in * Wi, 2],
                                                   [2 * Wi, nrc], [2, n]])
                                        last = (dx == 2 and dy == 2
                                                and cbp == cbi // 2 - 1)
                                        nc.tensor.matmul(
                                            o,
                                            wt[:, 2 * cbp:2 * cbp + 2,
                                               dy, dx, :],
                                            rhs, start=first, stop=last,
                                            perf_mode=DRM)
                                        first = False
                        else:
                            for dx in (1, 0, 2):
                                for cb in range(cbi):
                                    for dy in range(3):
                                        row0 = (2 * (y0 + ch * nrc) - 1
                                                + dy - i0)
                                        boff = cb * rows_in * Wi + row0 * Wi
                                        if dx == 0:
                                            o = _ap(pt[:, 0, 0], 1,
                                                    [[Wo, nrc], [1, Wo - 1]])
                                            rhs = _ap(sl[:, 0, 0, 0],
                                                      boff + 1,
                                                      [[2 * Wi, nrc],
                                                       [2, Wo - 1]])
                                        elif dx == 2 and Wo * 2 == Wi:
                                            o = pt
                                            rhs = _ap(sl[:, 0, 0, 0],
                                                      boff + 1,
                                                      [[2 * Wi, nrc],
                                                       [2, Wo]])
                                        else:
                                            o = pt
                                            rhs = _ap(sl[:, 0, 0, 0],
                                                      boff + dx - 1,
                                                      [[2 * Wi, nrc],
                                                       [2, Wo]])
                                        last = (dx == 2 and cb == cbi - 1
                                                and dy == 2)
                                        nc.tensor.matmul(
                                            o, wt[:, cb, dy, dx, :],
                                            rhs, start=first, stop=last)
                                        first = False
                        if slice_fn is None:
                            nc.vector.bn_stats(
                                out=stt[:, m, s * nchunk + ch, :],
                                in_=pt.rearrange("p a b -> p (a b)"))
                        else:
                            for (ei, (rlo, rhi)) in enumerate(slice_fn(ch)):
                                nc.vector.bn_stats(
                                    out=stt[:, m, ch * 2 + ei, :],
                                    in_=_ap(pt[:, 0, 0], rlo * Wo,
                                            [[1, (rhi - rlo) * Wo]]))
                        nc.scalar.activation(
                            out=osb[:, ch * nrc:(ch + 1) * nrc, :], in_=pt,
                            func=AF.Copy)
                    nc.sync.dma_start(
                        out=_ap(dst[m, 0:P, 0, 0], y0 * Wo,
                                [[Wo, nr], [1, Wo]]),
                        in_=osb)

            for s in range(nstrip + 2):
                if s < nstrip:
                    load(s)
                if s == min(1, nstrip - 1) and pre is not None:
                    pre()
                if s >= 2:
                    compute(s - 2)
                if 1 <= s < nstrip + 1:
                    relu(s - 1)
            return stt

        stt_d1 = down_layer(1, "h2", h1, h2, "h1", 1, 2, 144, 256, 12, 4,
                            pre=lambda: HOLD.__setitem__(
                                "h1", layer_stats("h1", stt0, 1, 72)))
        stt_d2 = down_layer(2, "h3", h2, h3, "h2", 2, 4, 72, 128, 12, 4,
                            pre=lambda: HOLD.__setitem__(
                                "h2", layer_stats("h2", stt_d1, 2, 18)))
        stt_d3 = down_layer(3, "h4", h3, h4, "h3", 4, 8, 36, 64, 18, 9,
                            slice_fn=slices_d3, single=True,
                            pre=lambda: HOLD.__setitem__(
                                "h3", layer_stats("h3", stt_d2, 4, 9)))

        # ================= up layers u0..u2 ================================
        nc.leave_named_scope("down", _sc, False)
        _sc = nc.enter_named_scope("up", False)[0]

        def up_layer(li, name, src, dst, in_name, cbi, cbo, Sin, Wi, idx_map,
                     single=False, pre=None):
            Mo = 128
            Wo = 2 * Wi
            Sout = 2 * Sin
            nstrip, subs, slfn = UP_SCHED[name]
            nro = Sout // nstrip  # out rows per strip
            nent = len(LAYER_ENTRIES[name])
            stt = stp.tile([P, cbo, nent, 6], F32, name=f"stu{li}",
                           tag="stats")
            slabs = [None] * nstrip
            srows = []
            for s in range(nstrip):
                y0 = s * nro
                i_lo = max((y0 - 1) // 2, 0)
                i_hi = min((y0 + nro) // 2 + 1, Sin)
                srows.append((i_lo, i_hi))
            rows_in = max(hi - lo for lo, hi in srows) + 1

            def load(s):
                i_lo, i_hi = srows[s]
                sl = sb.tile([P, cbi, rows_in, Wi], F8, name=f"slu{li}",
                             tag="inslabB" if single else "inslab",
                             bufs=1 if single else None)
                for cb in range(cbi):
                    nc.sync.dma_start(
                        out=sl[:, cb, 0:i_hi - i_lo, :],
                        in_=_ap(src[cb, 0:P, 0, 0], i_lo * Wi,
                                [[Wi, i_hi - i_lo], [1, Wi]]))
                slabs[s] = sl

            def relu(s):
                i_lo, i_hi = srows[s]
                sl = slabs[s]
                st_in = HOLD[in_name]
                for cb in range(cbi):
                    nc.scalar.activation(
                        out=sl[:, cb, 0:i_hi - i_lo, :],
                        in_=sl[:, cb, 0:i_hi - i_lo, :], func=AF.Relu,
                        bias=st_in[cb][:, 1:2], scale=st_in[cb][:, 0:1])
                if i_hi - i_lo < rows_in:
                    nc.vector.memset(sl[:, :, i_hi - i_lo:rows_in, :], 0.0)

            def compute(s, m, wt):
                i_lo, i_hi = srows[s]
                sl = slabs[s]
                y0 = s * nro
                osb = osl.tile([Mo, nro, Wo], F8, name=f"osu{li}",
                               tag="outslab")
                for a in range(2):
                    kys = [1] if a == 0 else [0, 2]
                    for b_ in range(2):
                        kxs = [1] if b_ == 0 else [2, 0]
                        k0 = 0
                        for su, rsub in enumerate(subs):
                            yb = y0 + a + 2 * k0
                            pt = ps.tile([Mo, rsub, Wi], F32, name=f"ptu{li}",
                                         tag="mm")
                            first = True
                            for kx in kxs:
                                j0 = (b_ + 1 - kx) // 2
                                trim = 1 if (kx == 0 and j0 == 1) else 0
                                n = Wi - 1 if trim else Wi
                                o = pt[:, :, 0:n] if trim else pt
                                for ky in kys:
                                    i_first = (yb + 1 - ky) // 2
                                    for cbp in range(cbi // 2):
                                        boff = (2 * cbp * rows_in * Wi
                                                + (i_first - i_lo) * Wi + j0)
                                        rhs = _ap(sl[:, 0, 0, 0], boff,
                                                  [[rows_in * Wi, 2],
                                                   [Wi, rsub], [1, n]])
                                        last = (kx == kxs[-1] and ky == kys[-1]
                                                and cbp == cbi // 2 - 1)
                                        nc.tensor.matmul(
                                            o,
                                            wt[:, 2 * cbp:2 * cbp + 2,
                                               ky, kx, :],
                                            rhs, start=first, stop=last,
                                            perf_mode=DRM)
                                        first = False
                            for (ei, (klo, khi)) in enumerate(slfn(s, su)):
                                nc.vector.bn_stats(
                                    out=stt[:, m, idx_map[(s, a, b_, su, ei)], :],
                                    in_=_ap(pt[:, 0, 0], klo * Wi,
                                            [[1, (khi - klo) * Wi]]))
                            oap = _ap(osb[:, 0, 0],
                                      (a + 2 * k0) * Wo + b_,
                                      [[2 * Wo, rsub], [2, Wi]])
                            nc.scalar.activation(out=oap, in_=pt,
                                                 func=AF.Copy)
                            k0 += rsub
                nc.sync.dma_start(
                    out=_ap(dst[m, 0:Mo, 0, 0], y0 * Wo, [[Wo, nro], [1, Wo]]),
                    in_=osb)

            for s in range(nstrip):
                load(s)
                if s == 0 and pre is not None:
                    pre()
                relu(s)
                for m in range(cbo):
                    wt = wpm.tile([P, cbi, 3, 3, Mo], F8, name=f"wtu{li}",
                                  tag="wup")
                    nc.sync.dma_start(out=wt, in_=wu[li][m])
                    compute(s, m, wt)
            return stt

        stt_u0 = up_layer(0, "g0", h4, g0, "h4", 8, 4, 18, 32, IDX_G0,
                          single=True,
                          pre=lambda: HOLD.__setitem__(
                              "h4", layer_stats("h4", stt_d3, 8, 4)))
        stt_u1 = up_layer(1, "g1", g0, g1, "g0", 4, 2, 36, 64, IDX_G1,
                          pre=lambda: HOLD.__setitem__(
                              "g0", layer_stats("g0", stt_u0, 4, 16)))
        stt_u2 = up_layer(2, "g2", g1, g2, "g1", 2, 1, 72, 128, IDX_G2,
                          pre=lambda: HOLD.__setitem__(
                              "g1", layer_stats("g1", stt_u1, 2, 32)))

        # ================= u3: 128 -> 64, M-packed into g3 =================
        wu3at = wsm.tile([128, 3, 128], BF16, name="wu3at")
        nc.sync.dma_start(out=wu3at, in_=wu3a[:, :, :])
        wu3bt = wsm.tile([128, 3, 64], BF16, name="wu3bt")
        nc.sync.dma_start(out=wu3bt, in_=wu3b[:, :, :])
        stt3 = stp.tile([P, 1, 144, 6], F32, name="stt3", tag="stats")
        NU3 = 6
        slabs3 = [None] * NU3

        def u3_load(s):
            q0 = s * 24
            i_hi = min(q0 + 25, 144)
            sl = sb.tile([P, 26, 256], F8, name="sl3", tag="inslab")
            nc.sync.dma_start(out=sl[:, 0:i_hi - q0, :],
                              in_=_ap(g2[0, 0:P, 0, 0], q0 * 256,
                                      [[256, i_hi - q0], [1, 256]]))
            slabs3[s] = (sl, i_hi - q0)

        def u3_relu(s):
            sl, nreal = slabs3[s]
            st_ = HOLD["g2"][0]
            cuts = [(0, 13), (13, nreal)] if s == 0 else [(0, nreal)]
            for (rl, rh) in cuts:
                nc.scalar.activation(out=sl[:, rl:rh, :],
                                     in_=sl[:, rl:rh, :],
                                     func=AF.Relu, bias=st_[:, 1:2],
                                     scale=st_[:, 0:1])
            if nreal < 26:
                nc.vector.memset(sl[:, nreal:26, :], 0.0)

        def u3_compute(s):
            sl, _n = slabs3[s]
            q0s = s * 24
            for blk in range(12):
                q0 = blk * 2  # local to slab
                pb0 = ps.tile([P, 2, 256], F32, name="pb0", tag="mm")
                pb1 = ps.tile([P, 2, 256], F32, name="pb1", tag="mm")
                # T1: full, start
                nc.tensor.matmul(pb0, wu3at[:, 0, :],
                                 _ap(sl[:, 0, 0], q0 * 256,
                                     [[256, 2], [1, 256]]),
                                 start=True, stop=False)
                # T2: rhs i=q+1, M 64..127
                nc.tensor.matmul(pb0[64:128, :, :], wu3bt[:, 0, :],
                                 _ap(sl[:, 0, 0], (q0 + 1) * 256,
                                     [[256, 2], [1, 256]]),
                                 start=False, stop=True)
                # T3: full, start
                nc.tensor.matmul(pb1, wu3at[:, 1, :],
                                 _ap(sl[:, 0, 0], q0 * 256,
                                     [[256, 2], [1, 256]]),
                                 start=True, stop=False)
                # T4: cols p+1, trim last
                nc.tensor.matmul(pb1[:, :, 0:255], wu3at[:, 2, :],
                                 _ap(sl[:, 0, 0], q0 * 256 + 1,
                                     [[256, 2], [1, 255]]),
                                 start=False, stop=False)
                # T5: rhs i=q+1 col p, M 64..127
                nc.tensor.matmul(pb1[64:128, :, :], wu3bt[:, 1, :],
                                 _ap(sl[:, 0, 0], (q0 + 1) * 256,
                                     [[256, 2], [1, 256]]),
                                 start=False, stop=False)
                # T6: rhs i=q+1 col p+1, M 64..127, trim
                nc.tensor.matmul(pb1[64:128, :, 0:255], wu3bt[:, 2, :],
                                 _ap(sl[:, 0, 0], (q0 + 1) * 256 + 1,
                                     [[256, 2], [1, 255]]),
                                 start=False, stop=True)
                eidx = (s * 12 + blk) * 2
                osb = osl.tile([P, 2, 518], F8, name="os3", tag="outslab")
                nc.scalar.activation(
                    out=_ap(osb[:, 0, 0], 3, [[518, 2], [2, 256]]), in_=pb0,
                    func=AF.Copy)
                nc.vector.tensor_copy(
                    out=_ap(osb[:, 0, 0], 4, [[518, 2], [2, 256]]), in_=pb1)
                nc.vector.bn_stats(out=stt3[:, 0, eidx, :],
                                   in_=pb0.rearrange("p a b -> p (a b)"))
                nc.vector.bn_stats(out=stt3[:, 0, eidx + 1, :],
                                   in_=pb1.rearrange("p a b -> p (a b)"))
                # g3 pair = q + 2 (pad4 top)
                nc.sync.dma_start(
                    out=_ap(g3[0:P, 0, 0], (q0s + q0 + 2) * 518,
                            [[518, 2], [1, 518]]),
                    in_=osb)
        for s in range(NU3 + 2):
            if s < NU3:
                u3_load(s)
            if s == 1:
                HOLD["g2"] = layer_stats("g2", stt_u2, 1, 84)
            if s >= 2:
                u3_compute(s - 2)
            if 1 <= s < NU3 + 1:
                u3_relu(s - 1)

        # --- g3 gutters: rows (reflect, partition-sliced) + cols ----------
        # padded row 1 <- 7, 2 <- 6, 3 <- 5 ; 292 <- 290, 293 <- 289, 294<-288
        for (d_, s_) in ((0, 8), (1, 7), (2, 6), (3, 5), (292, 290),
                         (293, 289), (294, 288), (295, 287)):
            dp, dj = divmod(d_, 2)
            sp, sj = divmod(s_, 2)
            nc.sync.dma_start(
                out=_ap(g3[dj * 64:(dj + 1) * 64, 0, 0], dp * 518, [[1, 518]]),
                in_=_ap(g3[sj * 64:(sj + 1) * 64, 0, 0], sp * 518, [[1, 518]]))
        # ================= final 7x7 + tanh + masked sum ===================
        nc.leave_named_scope("up", _sc, False)
        _sc = nc.enter_named_scope("final", False)[0]
        wfAt = wsm.tile([128, 7, 126], BF16, name="wfAt")
        nc.sync.dma_start(out=wfAt, in_=wfA[:, :, :])
        wfSt = wsm.tile([126, 7, 18], BF16, name="wfSt")
        nc.sync.dma_start(out=wfSt, in_=wfS[:, :, :])
        bft = nrm.tile([18, 1], F32, name="bft")
        nc.sync.dma_start(out=bft, in_=bfv[:, :])
        acc = nrm.tile([18, 96], F32, name="acc")
        nc.vector.memset(acc, 0.0)
        NF = 24  # strips of 12 output rows (2 y0-groups of 6)
        fslabs = [None] * NF

        def f_load(s):
            sl = sb.tile([128, 10, 518], F8, name="slF", tag="inslab")
            nc.sync.dma_start(out=sl, in_=_ap(g3[0:128, 0, 0], 6 * s * 518,
                                              [[518, 10], [1, 518]]))
            # reflect column gutters in SBUF (g3 cols 0..2/515..517 are junk)
            for k in range(3):
                nc.vector.tensor_copy(out=sl[:, :, k:k + 1],
                                      in_=sl[:, :, 6 - k:7 - k])
                nc.vector.tensor_copy(out=sl[:, :, 515 + k:516 + k],
                                      in_=sl[:, :, 513 - k:514 - k])
            mt = sb.tile([18, 4, 256], F32, name="mt", tag="mslab")
            nc.sync.dma_start(out=mt, in_=_dap(maskrep, s * 1024,
                                               [[48 * 512, 18], [1, 1024]]))
            fslabs[s] = (sl, mt)

        def f_relu(s):
            sl, _m = fslabs[s]
            st_ = HOLD["g3"][0]
            cuts = [(0, 7), (7, 10)] if s == 0 else [(0, 10)]
            for (rl, rh) in cuts:
                nc.scalar.activation(out=sl[:, rl:rh, :], in_=sl[:, rl:rh, :],
                                     func=AF.Relu,
                                     bias=st_[:, 1:2], scale=st_[:, 0:1])

        def f_compute(s):
            sl, mt = fslabs[s]
            for g in range(2):
                for hx in range(2):
                    ptA_f = psf.tile([128, 262], F32, name="ptA", tag="fa")
                    ptA = ptA_f[0:126, :]
                    for t in range(7):
                        rhs = _ap(sl[:, 0, 0], (3 * g + t) * 518 + hx * 256,
                                  [[1, 262]])
                        nc.tensor.matmul(ptA, wfAt[:, t, :], rhs,
                                         start=(t == 0), stop=(t == 6))
                    stg = osl.tile([126, 262], F8, name="stg", tag="fstg")
                    nc.vector.tensor_copy(out=stg, in_=ptA)
                    ptB_f = psf.tile([128, 256], F32, name="ptB", tag="fb", bufs=1)
                    ptB = ptB_f[0:18, :]
                    for dx in range(7):
                        nc.tensor.matmul(ptB, wfSt[:, dx, :],
                                         stg[:, dx:dx + 256],
                                         start=(dx == 0), stop=(dx == 6))
                    t1 = osl.tile([18, 256], F32, name="ft1", tag="ft1")
                    nc.vector.tensor_scalar(out=t1, in0=ptB, scalar1=bft,
                                            scalar2=None, op0=ALU.add)
                    nc.vector.tensor_mul(out=t1, in0=t1,
                                         in1=mt[:, 2 * g + hx, :])
                    th = osl.tile([18, 256], F32, name="fth", tag="fth")
                    ai = 4 * s + 2 * g + hx
                    nc.scalar.activation(out=th, in_=t1, func=AF.Tanh,
                                         accum_out=acc[:, ai:ai + 1])

        for s in range(NF + 2):
            if s < NF:
                f_load(s)
            if s == 1:
                HOLD["g3"] = layer_stats("g3", stt3, 1, 144, fold=True)
            if s >= 2:
                f_compute(s - 2)
            if 1 <= s < NF + 1:
                f_relu(s - 1)

        osum_t = nrm.tile([18, 1], F32, name="osum_t")
        nc.vector.tensor_reduce(out=osum_t, in_=acc, op=ALU.add,
                                axis=mybir.AxisListType.X)
        nc.sync.dma_start(out=osum[:, :], in_=osum_t)

        if debug:
            for nm, tens in [("h0", h0), ("g3", g3)]:
                sh = dbg[nm].shape
                nc.sync.dma_start(
                    out=_dap(dbg[nm], 0, [[sh[1], sh[0]], [1, sh[1]]]),
                    in_=_dap(tens, 0, [[sh[1], sh[0]], [1, sh[1]]]))
            for nm, tens, cbo, sz in [("h1", h1, 1, 144 * 256),
                                      ("h2", h2, 2, 72 * 128),
                                      ("h3", h3, 4, 36 * 64),
                                      ("h4", h4, 8, 18 * 32),
                                      ("g0", g0, 4, 36 * 64),
                                      ("g1", g1, 2, 72 * 128),
                                      ("g2", g2, 1, 144 * 256)]:
                for m in range(cbo):
                    nc.sync.dma_start(
                        out=_dap(dbg[nm], m * P * sz, [[sz, P], [1, sz]]),
                        in_=_dap(tens, m * P * sz, [[sz, P], [1, sz]]))
            for i, nm in enumerate(LAYER_ORDER):
                nc.sync.dma_start(out=dbg["st"][:, 2 * i:2 * i + 2],
                                  in_=HOLD[nm][0][:, :])
            off = 0
            for k in ["h0", "h1", "h2", "h3", "h4", "g0", "g1", "g2", "g3"]:
                n_ = CT[k]
                nc.sync.dma_start(out=dbg["sr"][off:off + n_, :],
                                  in_=stat_r[k][0:n_, :])
                nc.sync.dma_start(out=dbg["sl"][off:off + n_, :],
                                  in_=stat_l[k][:, :])
                off += n_

    nc.finalize()
    return nc


_CACHE = {}


def make_in_maps(inputs):
    wblobs = prep_weights(inputs)
    x = np.asarray(inputs["x"], np.float32)
    inst = np.asarray(inputs["inst"])
    return [prep_core_inputs(x[c // 2], inst[c // 2, 0], wblobs, c % 2)
            for c in range(8)]


def combine_outputs(res, inst):
    mask = (np.asarray(inst) == 1).astype(np.float32)  # [B,1,H,W]
    cnt = mask.sum((2, 3))  # [B,1]
    out = np.zeros((B, 3, H, W), np.float32)
    for b in range(B):
        s_top = res.results[2 * b]["osum"].reshape(6, 3).sum(0)
        s_bot = res.results[2 * b + 1]["osum"].reshape(6, 3).sum(0)
        mean = (s_top + s_bot) / cnt[b, 0]
        out[b] = mean[:, None, None] * mask[b, 0]
    return out


def kernel(**inputs):
    if "nc" not in _CACHE:
        _CACHE["nc"] = build_kernel()
    res = run_bass_kernel_spmd(_CACHE["nc"], make_in_maps(inputs),
                               core_ids=list(range(8)))
    return combine_outputs(res, inputs["inst"])



# revision 81
# speedup vs baseline: 1.1339x; 1.1282x over previous
# Trainium2 Bass kernel for nn_Encoder_81509889343552 — spatially sharded v2.
#
# Each image (B=4) is split top/bottom across a core pair (8 cores total).
# Uniform SPMD spans via exact doubling: per-core local row spans
#   h0=288, h1=144, h2=72, h3=36, h4=18, g0=36, g1=72, g2=144, g3=288, hf=288.
# InstanceNorm stats are exact: per-strip bn_stats entries are weighted by a
# host 0/1 ownership vector (each core owns its half of every layer), then a
# tiny per-layer AllReduce of (mean, var+mean^2) across the pair combines the
# halves. Junk rows near the interior boundary (from zero-pad instead of real
# neighbor data) are excluded from ownership by construction.
#
# bf16 activations/weights (f32 stats + PSUM), K-packed L1 (K=96), pair-packed
# h0 (d0 gets K=128+K=64 taps), M-packed u3 (row pairs in PE columns), fused
# masked segment mean (no hf DRAM roundtrip; tanh(mask*(x+b)) = mask*tanh(x+b)
# for a binary mask). Host combines the pair partial sums and scatters.
import sys

sys.path.insert(0, "/opt/trn_rl_repo")

import contextlib

import numpy as np
import ml_dtypes

import concourse.bass as bass
import concourse.bacc as bacc
import concourse.tile as tile
from concourse import mybir
from concourse.bass_utils import run_bass_kernel_spmd

F32 = mybir.dt.float32
F32R = mybir.dt.float32r
BF16 = mybir.dt.bfloat16
AF = mybir.ActivationFunctionType
ALU = mybir.AluOpType
BFH = ml_dtypes.bfloat16
F8 = mybir.dt.float8e4
F8H = ml_dtypes.float8_e4m3
DRM = mybir.MatmulPerfMode.DoubleRow

B, H, W = 4, 512, 512
EPS = 1e-5
P = 128
RG = [[0, 1], [2, 3], [4, 5], [6, 7]]

# local span / bottom base / owned-cutoff table (top cut = owned end,
# bot cut = owned start, in local rows)
SPAN = {
    "h0": (288, 224, 256, 32), "h1": (144, 112, 128, 16),
    "h2": (72, 56, 64, 8),     "h3": (36, 28, 32, 4),
    "h4": (18, 14, 16, 2),     "g0": (36, 28, 32, 4),
    "g1": (72, 56, 64, 8),     "g2": (144, 112, 128, 16),
    "g3": (288, 224, 256, 32),
}


def _ap(base, extra_off, dims):
    return bass.AP(
        tensor=base.tensor,
        offset=base.offset + extra_off,
        ap=[list(base.ap[0])] + [list(d) for d in dims],
    )


def _dap(handle, off, dims):
    return bass.AP(tensor=handle, offset=off, ap=[list(d) for d in dims])


# ---------------------------------------------------------------------------
# Stats-entry enumeration (shared between host weight gen and device build).
# Each entry is (row_lo, row_hi) half-open in local layer rows; the device
# emits bn_stats in exactly this order per channel-block.
# ---------------------------------------------------------------------------

def entries_L1():
    return [(2 * k, 2 * k + 2) for k in range(144)]


def entries_d0():
    return [(2 * e, 2 * e + 2) for e in range(72)]


def entries_down(nstrip, nchunk, nrc):
    out = []
    for s in range(nstrip):
        for c in range(nchunk):
            lo = s * nchunk * nrc + c * nrc
            out.append((lo, lo + nrc))
    return out


def slices_d3(chunk):
    # chunk 0 = rows 0..8, chunk 1 = rows 9..17; cutoffs at rows {2, 16}
    return [(0, 2), (2, 9)] if chunk == 0 else [(0, 7), (7, 9)]


def entries_d3():
    return [(0, 2), (2, 9), (9, 16), (16, 18)]


# --- up-layer su/slice schedules -------------------------------------------
# UP_SCHED[name] = (nstrip, subs_per_strip, slices(s, su) -> [(klo,khi)])
def _u0_slices(s, su):
    return [(0, 2), (2, 9)] if su == 0 else [(0, 7), (7, 9)]


def _u1_slices(s, su):
    if s == 0:
        return [(0, 4), (4, 8)] if su == 0 else \
            ([(0, 8)] if su == 1 else [(0, 2)])
    return [(0, 6), (6, 8)] if su == 1 else \
        ([(0, 8)] if su == 0 else [(0, 2)])


def _u2_slices(s, su):
    if s == 3 and su == 2:
        return [(0, 2), (2, 4)]
    return [(0, 4)] if su < 4 else [(0, 2)]


UP_SCHED = {
    "g0": (1, [9, 9], _u0_slices),
    "g1": (2, [8, 8, 2], _u1_slices),
    "g2": (4, [4, 4, 4, 4, 2], _u2_slices),
}


def up_entries_and_index(name, Sout):
    nstrip, subs, slfn = UP_SCHED[name]
    nro = Sout // nstrip
    ents, idx = [], {}
    for s in range(nstrip):
        y0 = s * nro
        for a in range(2):
            for b_ in range(2):
                for su in range(len(subs)):
                    k0 = sum(subs[:su])
                    for ei, (klo, khi) in enumerate(slfn(s, su)):
                        idx[(s, a, b_, su, ei)] = len(ents)
                        ents.append((y0 + a + 2 * (k0 + klo),
                                     y0 + a + 2 * (k0 + khi - 1) + 1))
    return ents, idx


ENT_G0, IDX_G0 = up_entries_and_index("g0", 36)
ENT_G1, IDX_G1 = up_entries_and_index("g1", 72)
ENT_G2, IDX_G2 = up_entries_and_index("g2", 144)


def entries_u3():
    # 6 strips x 12 blocks x 2 (pb0, pb1); block = 2 out-row pairs = 4 rows
    out = []
    for s in range(6):
        for blk in range(12):
            q0 = s * 24 + blk * 2
            out.append((2 * q0, 2 * q0 + 4))
            out.append((2 * q0, 2 * q0 + 4))
    return out


LAYER_ENTRIES = {
    "h0": entries_L1(), "h1": entries_d0(),
    "h2": entries_down(6, 3, 4), "h3": entries_down(3, 3, 4),
    "h4": entries_d3(), "g0": ENT_G0, "g1": ENT_G1,
    "g2": ENT_G2, "g3": entries_u3(),
}
LAYER_ORDER = ["h0", "h1", "h2", "h3", "h4", "g0", "g1", "g2", "g3"]
LAYER_CBO = {"h0": 1, "h1": 1, "h2": 2, "h3": 4, "h4": 8,
             "g0": 4, "g1": 2, "g2": 1, "g3": 1}


def statw_vector(half):
    """Concatenated per-entry 0/1 weights (x6 fields, replicated per
    channel-block) for this core half."""
    vals = []
    offs = {}
    for name in LAYER_ORDER:
        S, bbase, cut_top, cut_bot = SPAN[name]
        offs[name] = len(vals)
        lw = []
        for (lo, hi) in LAYER_ENTRIES[name]:
            if half == 0:
                w = 1.0 if hi <= cut_top else 0.0
                assert hi <= cut_top or lo >= cut_top, (name, lo, hi)
            else:
                w = 1.0 if lo >= cut_bot else 0.0
                assert lo >= cut_bot or hi <= cut_bot, (name, lo, hi)
            lw.extend([w] * 6)
        vals.extend(lw * LAYER_CBO[name])
    return np.asarray(vals, np.float32), offs


STATW_TOP, STATW_OFFS = statw_vector(0)
STATW_BOT, _ = statw_vector(1)
NSTATW = len(STATW_TOP)


# ---------------------------------------------------------------------------
# Host-side weight preprocessing (all lhsT blobs in SBUF layout, bf16)
# ---------------------------------------------------------------------------

def prep_weights(inp):
    w = {}
    # L1: K = (ci, dy, l4) = 96; M = r*64+co; matmul d in {0,1}: kx = 4d + l
    w0 = np.asarray(inp["w0"], np.float32)  # [64, 3, 7, 7]
    w1 = np.zeros((96, 2, 128), np.float32)
    for ci in range(3):
        for dy in range(8):
            for l in range(4):
                p = ci * 32 + dy * 4 + l
                for d in range(2):
                    kx = 4 * d + l
                    if kx > 6:
                        continue
                    for r in range(2):
                        ky = dy - r
                        if 0 <= ky <= 6:
                            w1[p, d, r * 64:(r + 1) * 64] = w0[:, ci, ky, kx]
    w["w1"] = w1.astype(BFH)

    # d0 (pair-packed h0): K128 tap (pair y): rows (j,c): ky=1+j;
    # K64 tap (pair y-1, j=1 partitions 64..127): ky=0.
    dw0 = np.asarray(inp["dw0"], np.float32)  # [128, 64, 3, 3]
    wd0a = np.zeros((128, 3, 128), np.float32)
    wd0b = np.zeros((128, 3, 128), np.float32)
    for dx in range(3):
        for j in range(2):
            for c in range(64):
                wd0a[j * 64 + c, dx, :] = dw0[:, c, 1 + j, dx]
        for c in range(64):
            wd0b[64 + c, dx, :] = dw0[:, c, 0, dx]
    w["wd0a"] = wd0a.astype(BFH)
    w["wd0b"] = wd0b.astype(BFH)

    # d1..d3: [cbo, K=128, cbi, 3, 3, M=128] (k-major per m-block)
    c = 128
    for i in (1, 2, 3):
        dw = np.asarray(inp[f"dw{i}"], np.float32)  # [2c, c, 3, 3]
        cbo, cbi = (2 * c) // P, c // P
        ndy = 4 if i == 1 else 3
        blob = np.zeros((cbo, P, cbi, ndy, 3, P), np.float32)
        for m in range(cbo):
            for cb in range(cbi):
                for dy in range(3):
                    for dx in range(3):
                        blob[m, :, cb, dy, dx, :] = \
                            dw[m * P:(m + 1) * P, cb * P:(cb + 1) * P, dy, dx].T
        w[f"wd{i}"] = blob.astype(F8H)
        c *= 2

    # u0..u2 (torch convT layout uw [Cin, Cout, 3, 3]):
    # [cbo, K=128, cbi, 3, 3, Mo]
    for i in (0, 1, 2):
        uw = np.asarray(inp[f"uw{i}"], np.float32)
        Cin_, Cout_ = uw.shape[0], uw.shape[1]
        cbi, cbo, Mo = Cin_ // P, max(Cout_ // P, 1), min(Cout_, P)
        blob = np.zeros((cbo, P, cbi, 3, 3, Mo), np.float32)
        for m in range(cbo):
            for cb in range(cbi):
                for ky in range(3):
                    for kx in range(3):
                        blob[m, :, cb, ky, kx, :] = \
                            uw[cb * P:(cb + 1) * P, m * Mo:(m + 1) * Mo, ky, kx]
        w[f"wu{i}"] = blob.astype(F8H)

    # u3 M-packed: psum partition q = j*64 + c (j = out row parity).
    # T1 (b0, rhs i=q,  col p):  j0: (ky1,kx1); j1: (ky2,kx1)
    # T2 (b0, rhs i=q+1,col p):  j1: (ky0,kx1)   [M 64..127]
    # T3 (b1, rhs i=q,  col p):  j0: (ky1,kx2); j1: (ky2,kx2)
    # T4 (b1, rhs i=q,  col p+1):j0: (ky1,kx0); j1: (ky2,kx0)
    # T5 (b1, rhs i=q+1,col p):  j1: (ky0,kx2)
    # T6 (b1, rhs i=q+1,col p+1):j1: (ky0,kx0)
    uw3 = np.asarray(inp["uw3"], np.float32)  # [128, 64, 3, 3]
    wa = np.zeros((128, 3, 128), np.float32)  # T1, T3, T4
    wb = np.zeros((128, 3, 64), np.float32)   # T2, T5, T6
    for t, (ky0_, kx0_, ky1_, kx1_) in enumerate(
            [(1, 1, 2, 1), (1, 2, 2, 2), (1, 0, 2, 0)]):
        wa[:, t, 0:64] = uw3[:, :, ky0_, kx0_]
        wa[:, t, 64:128] = uw3[:, :, ky1_, kx1_]
    for t, (ky_, kx_) in enumerate([(0, 1), (0, 2), (0, 0)]):
        wb[:, t, :] = uw3[:, :, ky_, kx_]
    w["wu3a"] = wa.astype(BFH)
    w["wu3b"] = wb.astype(BFH)

    # final conv stage A: K = (j, c) over 4 g3-pairs; slab pair t holds padded
    # rows 6k+2t+j for strip y0=6k; out row y0+r reads padded y0+1+ky'
    # (pad offset 4 at top => padded row == local row + 4... see g3 layout):
    # tap ky' = 2t + j - r - 1; M = r*21 + dx*3 + co.
    wf = np.asarray(inp["wf"], np.float32)  # [3, 64, 7, 7]
    wfA = np.zeros((128, 7, 126), np.float32)
    for t in range(7):
        for j in range(2):
            for r in range(6):
                ky = 2 * t + j - r - 1
                if 0 <= ky <= 6:
                    for dx in range(7):
                        for co in range(3):
                            wfA[j * 64:(j + 1) * 64, t, r * 21 + dx * 3 + co] = \
                                wf[co, :, ky, dx]
    w["wfA"] = wfA.astype(BFH)
    wfS = np.zeros((126, 7, 18), np.float32)
    for dx in range(7):
        for r in range(6):
            for co in range(3):
                wfS[r * 21 + dx * 3 + co, dx, r * 3 + co] = 1.0
    w["wfS"] = wfS.astype(BFH)
    bf = np.asarray(inp["bf"], np.float32)
    w["bfv"] = np.tile(bf, 6).reshape(18, 1).astype(np.float32)

    # j-fold (average partitions c and c+64) for h0 / g3 stats
    wfold = np.zeros((128, 64), np.float32)
    for j in range(2):
        for c_ in range(64):
            wfold[j * 64 + c_, c_] = 0.5
    w["wfold"] = wfold
    return w


def prep_core_inputs(x_img, inst_img, wblobs, half):
    """Per-core inputs: xrep (K-packed padded x slice), maskrep, statw."""
    xpad = np.pad(np.asarray(x_img, np.float32), ((0, 0), (3, 3), (3, 3)),
                  mode="reflect")  # [3, 518, 518]
    r0 = 0 if half == 0 else 224
    xrep = np.zeros((96, 288, 518), np.float32)
    for ci in range(3):
        for dy in range(8):
            for l in range(4):
                p = ci * 32 + dy * 4 + l
                hi = min(r0 + dy + 288, 518)
                rows = xpad[ci, r0 + dy:hi, :]
                xrep[p, :hi - (r0 + dy), :518 - l] = rows[:, l:]
    mask = (np.asarray(inst_img) == 1).astype(np.float32)  # [512, 512]
    base = 0 if half == 0 else 224
    maskrep = np.zeros((18, 48, 2, 256), np.float32)
    for s in range(48):
        for r in range(6):
            y = 6 * s + r
            gy = base + y
            owned = (y < 256) if half == 0 else (y >= 32)
            if owned:
                row = mask[gy]
                for co in range(3):
                    maskrep[r * 3 + co, s, 0, :] = row[:256]
                    maskrep[r * 3 + co, s, 1, :] = row[256:]
    m = {
        "xrep": xrep.astype(F8H),
        "maskrep": maskrep.reshape(18, 48 * 2 * 256),
        "statw": STATW_TOP if half == 0 else STATW_BOT,
    }
    m.update(wblobs)
    return m


# ---------------------------------------------------------------------------
# Device kernel
# ---------------------------------------------------------------------------

def build_kernel(debug=False):
    nc = bacc.Bacc(None, target_bir_lowering=False, num_devices=8)

    xrep = nc.dram_tensor("xrep", [96, 288, 518], F8, kind="ExternalInput")
    maskrep = nc.dram_tensor("maskrep", [18, 48 * 2 * 256], F32,
                             kind="ExternalInput")
    statw = nc.dram_tensor("statw", [NSTATW], F32, kind="ExternalInput")
    w1 = nc.dram_tensor("w1", [96, 2, 128], BF16, kind="ExternalInput")
    wd0a = nc.dram_tensor("wd0a", [128, 3, 128], BF16, kind="ExternalInput")
    wd0b = nc.dram_tensor("wd0b", [128, 3, 128], BF16, kind="ExternalInput")
    wd = {}
    c = 128
    for i in (1, 2, 3):
        cbo, cbi = (2 * c) // P, c // P
        wd[i] = nc.dram_tensor(f"wd{i}",
                               [cbo, P, cbi, 4 if i == 1 else 3, 3, P], F8,
                               kind="ExternalInput")
        c *= 2
    wu = {}
    c = 1024
    for i in (0, 1, 2):
        cbi, cbo, Mo = c // P, max((c // 2) // P, 1), min(c // 2, P)
        wu[i] = nc.dram_tensor(f"wu{i}", [cbo, P, cbi, 3, 3, Mo], F8,
                               kind="ExternalInput")
        c //= 2
    wu3a = nc.dram_tensor("wu3a", [128, 3, 128], BF16, kind="ExternalInput")
    wu3b = nc.dram_tensor("wu3b", [128, 3, 64], BF16, kind="ExternalInput")
    wfA = nc.dram_tensor("wfA", [128, 7, 126], BF16, kind="ExternalInput")
    wfS = nc.dram_tensor("wfS", [126, 7, 18], BF16, kind="ExternalInput")
    bfv = nc.dram_tensor("bfv", [18, 1], F32, kind="ExternalInput")
    wfold = nc.dram_tensor("wfold", [128, 64], F32, kind="ExternalInput")

    h0 = nc.dram_tensor("h0", [128, 144, 512], F8)  # pair-packed (j,c)
    h1 = nc.dram_tensor("h1", [1, 128, 144, 256], F8)
    h2 = nc.dram_tensor("h2", [2, 128, 72, 128], F8)
    h3 = nc.dram_tensor("h3", [4, 128, 36, 64], F8)
    h4 = nc.dram_tensor("h4", [8, 128, 18, 32], F8)
    g0 = nc.dram_tensor("g0", [4, 128, 36, 64], F8)
    g1 = nc.dram_tensor("g1", [2, 128, 72, 128], F8)
    g2 = nc.dram_tensor("g2", [1, 128, 144, 256], F8)
    g3 = nc.dram_tensor("g3", [128, 148, 518], F8)  # pair-packed, pad4 top
    # per-layer stats scratch in DRAM + allreduced copy
    CT = {"h0": 64, "h1": 128, "h2": 256, "h3": 512, "h4": 1024,
          "g0": 512, "g1": 256, "g2": 128, "g3": 64}
    stat_l = {k: nc.dram_tensor(f"stl_{k}", [v, 2], F32)
              for k, v in CT.items()}
    stat_r = {k: nc.dram_tensor(f"str_{k}", [2 * v, 2], F32)
              for k, v in CT.items()}
    osum = nc.dram_tensor("osum", [18, 1], F32, kind="ExternalOutput")

    dbg = {}
    if debug:
        for nm, sh in [("h0", [128, 144 * 512]), ("h1", [128, 144 * 256]),
                       ("h2", [256, 72 * 128]), ("h3", [512, 36 * 64]),
                       ("h4", [1024, 18 * 32]), ("g0", [512, 36 * 64]),
                       ("g1", [256, 72 * 128]), ("g2", [128, 144 * 256]),
                       ("g3", [128, 148 * 518])]:
            dbg[nm] = nc.dram_tensor("dbg_" + nm, sh, BF16,
                                     kind="ExternalOutput")
        dbg["st"] = nc.dram_tensor("dbg_st", [128, 2 * 9], F32,
                                   kind="ExternalOutput")
        dbg["sr"] = nc.dram_tensor("dbg_sr", [sum(CT.values()), 2], F32,
                                   kind="ExternalOutput")
        dbg["sl"] = nc.dram_tensor("dbg_sl", [sum(CT.values()), 2], F32,
                                   kind="ExternalOutput")
        dbg["stt1"] = nc.dram_tensor("dbg_stt1", [128, 144 * 6], F32,
                                     kind="ExternalOutput")

    with tile.TileContext(nc) as tc, contextlib.ExitStack() as ctx:
        sb = ctx.enter_context(tc.tile_pool(name="sb", bufs=3))
        osl = ctx.enter_context(tc.tile_pool(name="osl", bufs=2))
        wsm = ctx.enter_context(tc.tile_pool(name="wsm", bufs=1))
        wpm = ctx.enter_context(tc.tile_pool(name="wpm", bufs=2))
        nrm = ctx.enter_context(tc.tile_pool(name="nrm", bufs=1))
        stp = ctx.enter_context(tc.tile_pool(name="stp", bufs=1))
        ps = ctx.enter_context(tc.tile_pool(name="ps", bufs=3, space="PSUM"))
        psf = ctx.enter_context(tc.tile_pool(name="psf", bufs=2, space="PSUM"))

        eps_t = nrm.tile([P, 1], F32, name="eps_t")
        nc.vector.memset(eps_t, EPS)
        wfoldt = nrm.tile([P, 64], F32, name="wfoldt")
        nc.sync.dma_start(out=wfoldt, in_=wfold[:, :])

        # broadcast per-entry stat weights once: [128, NSTATW]
        # (NSTATW ~ 3.5k floats -> 14KB/partition; fine)
        wst_t = nrm.tile([P, NSTATW], F32, name="wst_t")
        nc.gpsimd.dma_start(out=wst_t,
                            in_=_dap(statw, 0, [[0, P], [1, NSTATW]]))

        st_tiles = {}
        HOLD = {}

        def layer_stats(name, stt, cbo, nent, fold=False):
            """stt [128, cbo, nent, 6] -> list of [128, 2] (scale, bias) APs
            per channel block. Weighted raw sums (NaN-proof, exact), batched
            over channel blocks; pairwise AllReduce of (mean, E)."""
            off = STATW_OFFS[name]
            n6 = cbo * nent * 6
            wl = {"h0": 512, "h1": 256, "h2": 128, "h3": 64, "h4": 32,
                  "g0": 64, "g1": 128, "g2": 256, "g3": 512}[name]
            npart = (wl // 2) * wl // (2 if fold else 1)
            tw = stp.tile([P, cbo, nent, 6], F32, name=f"tw_{name}", tag="tw")
            nc.vector.tensor_mul(
                out=tw.rearrange("p a b c -> p (a b c)"),
                in0=stt.rearrange("p a b c -> p (a b c)"),
                in1=wst_t[:, off:off + n6])
            cm = stp.tile([P, cbo, nent, 2], F32, name=f"cm_{name}", tag="cm")
            nc.vector.tensor_mul(
                out=cm.rearrange("p a b c -> p (a b c)"),
                in0=_ap(tw[:, 0, 0, 0], 0, [[6, cbo * nent], [3, 2]]),
                in1=_ap(tw[:, 0, 0, 0], 1, [[6, cbo * nent], [3, 2]]))
            e2 = stp.tile([P, cbo, nent, 2], F32, name=f"e2_{name}", tag="e2")
            nc.vector.tensor_mul(
                out=e2.rearrange("p a b c -> p (a b c)"),
                in0=cm.rearrange("p a b c -> p (a b c)"),
                in1=_ap(tw[:, 0, 0, 0], 1, [[6, cbo * nent], [3, 2]]))
            nc.vector.tensor_add(
                out=e2.rearrange("p a b c -> p (a b c)"),
                in0=e2.rearrange("p a b c -> p (a b c)"),
                in1=_ap(tw[:, 0, 0, 0], 2, [[6, cbo * nent], [3, 2]]))
            s1 = stp.tile([P, cbo], F32, name=f"s1_{name}", tag="s1")
            nc.vector.tensor_reduce(out=s1,
                                    in_=cm.rearrange("p a b c -> p a (b c)"),
                                    op=ALU.add, axis=mybir.AxisListType.X)
            s2 = stp.tile([P, cbo], F32, name=f"s2_{name}", tag="s2")
            nc.vector.tensor_reduce(out=s2,
                                    in_=e2.rearrange("p a b c -> p a (b c)"),
                                    op=ALU.add, axis=mybir.AxisListType.X)
            me = stp.tile([P, cbo, 2], F32, name=f"me_{name}", tag="me")
            nc.vector.tensor_scalar(out=me[:, :, 0:1], in0=s1,
                                    scalar1=1.0 / npart, scalar2=None,
                                    op0=ALU.mult)
            nc.vector.tensor_scalar(out=me[:, :, 1:2], in0=s2,
                                    scalar1=1.0 / npart, scalar2=None,
                                    op0=ALU.mult)
            if fold:
                pm = psf.tile([64, 2], F32, name=f"pm_{name}", tag="mini",
                              bufs=1)
                nc.tensor.matmul(pm, wfoldt, me[:, 0, :], start=True,
                                 stop=True)
                mef = stp.tile([64, 2], F32, name=f"mef_{name}", tag="mef")
                nc.scalar.activation(out=mef, in_=pm, func=AF.Copy)
                nc.gpsimd.dma_start(out=stat_l[name][0:64, :], in_=mef)
            else:
                nc.gpsimd.dma_start(
                    out=_dap(stat_l[name], 0, [[2, P], [256, cbo], [1, 2]]),
                    in_=me)
            nc.gpsimd.collective_compute(
                "AllGather", ALU.bypass, RG,
                ins=[stat_l[name][:, :]], outs=[stat_r[name][:, :]])
            # result loads go on the scalar queue so slab loads on sync are
            # not head-of-line blocked behind the collective
            ct_ = CT[name]
            lr = stp.tile([P, cbo, 2], F32, name=f"lr_{name}", tag="lr")
            lrb = stp.tile([P, cbo, 2], F32, name=f"lrb_{name}", tag="lrb")
            if fold:
                nc.scalar.dma_start(out=lr[0:64, 0, :],
                                    in_=stat_r[name][0:64, :])
                nc.sync.dma_start(out=lr[64:128, 0, :],
                                    in_=stat_r[name][0:64, :])
                nc.scalar.dma_start(out=lrb[0:64, 0, :],
                                    in_=stat_r[name][64:128, :])
                nc.gpsimd.dma_start(out=lrb[64:128, 0, :],
                                    in_=stat_r[name][64:128, :])
            else:
                nc.scalar.dma_start(
                    out=lr,
                    in_=_dap(stat_r[name], 0, [[2, P], [256, cbo], [1, 2]]))
                nc.gpsimd.dma_start(
                    out=lrb,
                    in_=_dap(stat_r[name], 2 * ct_,
                             [[2, P], [256, cbo], [1, 2]]))
            nc.vector.tensor_add(out=lr.rearrange("p a b -> p (a b)"),
                                 in0=lr.rearrange("p a b -> p (a b)"),
                                 in1=lrb.rearrange("p a b -> p (a b)"))
            t0 = stp.tile([P, cbo, 2], F32, name=f"t0_{name}", tag="t0")
            nc.vector.tensor_scalar(out=t0.rearrange("p a b -> p (a b)"),
                                    in0=lr.rearrange("p a b -> p (a b)"),
                                    scalar1=0.5, scalar2=None, op0=ALU.mult)
            mview = _ap(t0[:, 0, 0], 0, [[2, cbo]])
            eview = _ap(t0[:, 0, 0], 1, [[2, cbo]])
            var = stp.tile([P, cbo], F32, name=f"var_{name}", tag="var")
            nc.vector.tensor_mul(out=var, in0=mview, in1=mview)
            nc.vector.tensor_sub(out=var, in0=eview, in1=var)
            sd = stp.tile([P, cbo], F32, name=f"sd_{name}", tag="sd")
            nc.scalar.activation(out=sd, in_=var, func=AF.Sqrt,
                                 bias=eps_t, scale=1.0)
            stD = nrm.tile([P, cbo, 2], F32, name=f"st_{name}",
                           tag=f"st_{name}")
            nc.vector.reciprocal(out=_ap(stD[:, 0, 0], 0, [[2, cbo]]),
                                 in_=sd)
            nc.vector.tensor_mul(out=_ap(stD[:, 0, 0], 1, [[2, cbo]]),
                                 in0=mview,
                                 in1=_ap(stD[:, 0, 0], 0, [[2, cbo]]))
            nc.vector.tensor_scalar(out=_ap(stD[:, 0, 0], 1, [[2, cbo]]),
                                    in0=_ap(stD[:, 0, 0], 1, [[2, cbo]]),
                                    scalar1=-1.0, scalar2=None, op0=ALU.mult)
            outs = [stD[:, m, :] for m in range(cbo)]
            st_tiles[name] = outs
            return outs

        # ================= L1: 7x7 conv, 3 -> 64 (K=96) =====================
        _sc = nc.enter_named_scope("L1", False)[0]
        w1t = wsm.tile([96, 2, 128], BF16, name="w1t")
        nc.sync.dma_start(out=w1t, in_=w1[:, :, :])
        stt1 = stp.tile([P, 1, 144, 6], F32, name="stt1", tag="stats")
        NS1 = 36
        slabs1 = [None] * NS1

        def l1_load(s):
            sl = sb.tile([96, 8, 518], F8, name="sl1", tag="inslab")
            nc.sync.dma_start(out=sl, in_=_ap(xrep[0:96, 0, 0], s * 8 * 518,
                                              [[518, 8], [1, 518]]))
            slabs1[s] = sl

        def l1_compute(s):
            sl = slabs1[s]
            osb = osl.tile([P, 4, 512], F8, name="os1", tag="outslab")
            for k in range(4):
                pt = ps.tile([P, 512], F32, name="pt1", tag="mm")
                for d in range(2):
                    rhs = _ap(sl[:, 0, 0], 2 * k * 518 + 4 * d, [[1, 512]])
                    nc.tensor.matmul(pt, w1t[:, d, :], rhs,
                                     start=(d == 0), stop=(d == 1))
                nc.scalar.activation(out=osb[:, k, :], in_=pt, func=AF.Copy)
                nc.vector.bn_stats(out=stt1[:, 0, s * 4 + k, :],
                                   in_=osb[:, k, :])
            nc.sync.dma_start(out=_ap(h0[0:128, 0, 0], s * 4 * 512,
                                      [[512, 4], [1, 512]]),
                              in_=osb)

        for s in range(NS1 + 2):
            if s < NS1:
                l1_load(s)
            if s >= 2:
                l1_compute(s - 2)
        if debug:
            nc.sync.dma_start(out=dbg["stt1"][:, :],
                              in_=stt1.rearrange("p a b c -> p (a b c)"))

        # ================= d0: 3x3 s2, 64 -> 128 (pair-packed) =============
        nc.leave_named_scope("L1", _sc, False)
        _sc = nc.enter_named_scope("down", False)[0]
        wd0at = wsm.tile([128, 3, 128], BF16, name="wd0at")
        nc.sync.dma_start(out=wd0at, in_=wd0a[:, :, :])
        wd0bt = wsm.tile([128, 3, 128], BF16, name="wd0bt")
        nc.sync.dma_start(out=wd0bt, in_=wd0b[:, :, :])
        stt0 = stp.tile([P, 1, 72, 6], F32, name="stt0", tag="stats")
        ND0 = 36
        slabs0 = [None] * ND0

        def d0_load(s):
            y0 = s * 4
            sl = sb.tile([128, 5, 512], F8, name="sl0", tag="inslab")
            p_lo = max(y0 - 1, 0)
            nc.sync.dma_start(
                out=sl[:, p_lo - (y0 - 1):5, :],
                in_=_ap(h0[0:128, 0, 0], p_lo * 512,
                        [[512, 5 - (p_lo - (y0 - 1))], [1, 512]]))
            slabs0[s] = sl

        def d0_relu(s):
            sl = slabs0[s]
            st_ = HOLD["h0"][0]
            y0 = s * 4
            lo = 1 if y0 == 0 else 0
            cuts = [(lo, 3), (3, 5)] if s == 0 else [(lo, 5)]
            for (rl, rh) in cuts:
                nc.scalar.activation(out=sl[:, rl:rh, :],
                                     in_=sl[:, rl:rh, :],
                                     func=AF.Relu, bias=st_[:, 1:2],
                                     scale=st_[:, 0:1])
            if y0 == 0:
                nc.vector.memset(sl[:, 0:1, :], 0.0)

        def d0_compute(s):
            sl = slabs0[s]
            y0 = s * 4
            for ch in range(2):
                pt = ps.tile([P, 2, 256], F32, name="pt0", tag="mm")
                yb = 2 * ch  # local out row in strip
                first = True
                for dx in (1, 0, 2):
                    # K128 taps (pairs y), PE tile position (0, 0)
                    if dx == 0:
                        o = _ap(pt[:, 0, 0], 1, [[256, 2], [1, 255]])
                        rhs = _ap(sl[:, 0, 0], (yb + 1) * 512 + 1,
                                  [[512, 2], [2, 255]])
                    else:
                        o = pt
                        rhs = _ap(sl[:, 0, 0], (yb + 1) * 512 + dx - 1,
                                  [[512, 2], [2, 256]])
                    nc.tensor.matmul(o, wd0at[:, dx, :], rhs, start=first,
                                     stop=False)
                    first = False
                for dx in (1, 0, 2):
                    # K64 taps (ky=0, pairs y-1, j=1 half), position (64, 0)
                    if dx == 0:
                        o = _ap(pt[:, 0, 0], 1, [[256, 2], [1, 255]])
                        rhs = _ap(sl[64:128, 0, 0], yb * 512 + 1,
                                  [[512, 2], [2, 255]])
                    else:
                        o = pt
                        rhs = _ap(sl[64:128, 0, 0], yb * 512 + dx - 1,
                                  [[512, 2], [2, 256]])
                    nc.tensor.matmul(o, wd0bt[64:128, dx, :], rhs,
                                     start=False, stop=(dx == 2))
                nc.vector.bn_stats(out=stt0[:, 0, (y0 + yb) // 2, :],
                                   in_=pt.rearrange("p a b -> p (a b)"))
                osb = osl.tile([P, 2, 256], F8, name="os0", tag="outslab")
                nc.scalar.activation(out=osb, in_=pt, func=AF.Copy)
                nc.sync.dma_start(
                    out=_ap(h1[0, 0:128, 0, 0], (y0 + yb) * 256,
                            [[256, 2], [1, 256]]),
                    in_=osb)

        for s in range(ND0 + 2):
            if s < ND0:
                d0_load(s)
            if s == 1:
                HOLD["h0"] = layer_stats("h0", stt1, 1, 144, fold=True)
            if s >= 2:
                d0_compute(s - 2)
            if 1 <= s < ND0 + 1:
                d0_relu(s - 1)

        # ================= generic down layers d1..d3 ======================
        def down_layer(li, name, src, dst, in_name, cbi, cbo, Sin, Wi, nr,
                       nrc, slice_fn=None, single=False, pre=None):
            Wo = Wi // 2
            Sout = Sin // 2
            nstrip = Sout // nr
            nchunk = nr // nrc
            nent = len(LAYER_ENTRIES[name])
            stt = stp.tile([P, cbo, nent, 6], F32, name=f"std{li}",
                           tag="stats")
            rows_in = 2 * nr + 1
            slabs = [None] * nstrip

            # li==1 gets one extra zeroed pad row for its zero dy DR tap
            rpad = 1 if li == 1 else 0

            def load(s):
                y0 = s * nr
                i0 = 2 * y0 - 1
                lo = max(i0, 0)
                sl = sb.tile([P, cbi, rows_in + rpad, Wi], F8,
                             name=f"sld{li}",
                             tag="inslabB" if single else "inslab",
                             bufs=1 if single else None)
                for cb in range(cbi):
                    nc.sync.dma_start(
                        out=sl[:, cb, lo - i0:rows_in, :],
                        in_=_ap(src[cb, 0:P, 0, 0], lo * Wi,
                                [[Wi, rows_in - (lo - i0)], [1, Wi]]))
                if rpad:
                    nc.vector.memset(sl[:, :, rows_in:, :], 0.0)
                slabs[s] = sl

            def relu(s):
                sl = slabs[s]
                st_in = HOLD[in_name]
                y0 = s * nr
                lo = 1 if y0 == 0 else 0
                cuts = ([(lo, 2 * nrc + 2), (2 * nrc + 2, rows_in)]
                        if s == 0 else [(lo, rows_in)])
                for cb in range(cbi):
                    for (rl, rh) in cuts:
                        nc.scalar.activation(
                            out=sl[:, cb, rl:rh, :],
                            in_=sl[:, cb, rl:rh, :], func=AF.Relu,
                            bias=st_in[cb][:, 1:2], scale=st_in[cb][:, 0:1])
                if y0 == 0:
                    nc.vector.memset(sl[:, :, 0:1, :], 0.0)

            def compute(s):
                sl = slabs[s]
                y0 = s * nr
                i0 = 2 * y0 - 1
                for m in range(cbo):
                    wt = wpm.tile([P, cbi, 4 if li == 1 else 3, 3, P], F8,
                                  name=f"wtd{li}", tag="wup")
                    nc.sync.dma_start(out=wt, in_=wd[li][m])
                    osb = osl.tile([P, nr, Wo], F8, name=f"osd{li}",
                                   tag="outslab")
                    for ch in range(nchunk):
                        pt = ps.tile([P, nrc, Wo], F32, name=f"ptd{li}",
                                     tag="mm")
                        first = True
                        if cbi >= 2:
                            # fp8 weights: every tap cb-paired -> pure-DR
                            # accumulation group
                            for dx in (1, 0, 2):
                                coloff = 0 if dx == 1 else 1
                                n = Wo - 1 if dx == 0 else Wo
                                o = (pt if dx != 0 else
                                     _ap(pt[:, 0, 0], 1,
                                         [[Wo, nrc], [1, n]]))
                                rowb = 2 * (y0 + ch * nrc) - 1 - i0
                                for cbp in range(cbi // 2):
                                    for dy in range(3):
                                        boff = (2 * cbp * rows_in * Wi
                                                + (rowb + dy) * Wi + coloff)
                                        rhs = _ap(sl[:, 0, 0, 0], boff,
                                                  [[rows_in * Wi, 2],
                                                   [2 * Wi, nrc], [2, n]])
                                        last = (dx == 2 and dy == 2
                                                and cbp == cbi // 2 - 1)
                                        nc.tensor.matmul(
                                            o,
                                            wt[:, 2 * cbp:2 * cbp + 2,
                                               dy, dx, :],
                                            rhs, start=first, stop=last,
                                            perf_mode=DRM)
                                        first = False
                        else:
                            # cbi==1 (d1): dy pairs (0,1) and (2, zero-pad 3)
                            for dx in (1, 0, 2):
                                coloff = 0 if dx == 1 else 1
                                n = Wo - 1 if dx == 0 else Wo
                                o = (pt if dx != 0 else
                                     _ap(pt[:, 0, 0], 1,
                                         [[Wo, nrc], [1, n]]))
                                rowb = 2 * (y0 + ch * nrc) - 1 - i0
                                for dyp in range(2):
                                    boff = (rowb + 2 * dyp) * Wi + coloff
                                    rhs = _ap(sl[:, 0, 0, 0], boff,
                                              [[Wi, 2], [2 * Wi, nrc],
                                               [2, n]])
                                    last = (dx == 2 and dyp == 1)
                                    nc.tensor.matmul(
                                        o, wt[:, 0, 2 * dyp:2 * dyp + 2,
                                              dx, :],
                                        rhs, start=first, stop=last,
                                        perf_mode=DRM)
                                    first = False
                        if slice_fn is None:
                            nc.vector.bn_stats(
                                out=stt[:, m, s * nchunk + ch, :],
                                in_=pt.rearrange("p a b -> p (a b)"))
                        else:
                            for (ei, (rlo, rhi)) in enumerate(slice_fn(ch)):
                                nc.vector.bn_stats(
                                    out=stt[:, m, ch * 2 + ei, :],
                                    in_=_ap(pt[:, 0, 0], rlo * Wo,
                                            [[1, (rhi - rlo) * Wo]]))
                        nc.scalar.activation(
                            out=osb[:, ch * nrc:(ch + 1) * nrc, :], in_=pt,
                            func=AF.Copy)
                    nc.sync.dma_start(
                        out=_ap(dst[m, 0:P, 0, 0], y0 * Wo,
                                [[Wo, nr], [1, Wo]]),
                        in_=osb)

            for s in range(nstrip + 2):
                if s < nstrip:
                    load(s)
                if s == min(1, nstrip - 1) and pre is not None:
                    pre()
                if s >= 2:
                    compute(s - 2)
                if 1 <= s < nstrip + 1:
                    relu(s - 1)
            return stt

        stt_d1 = down_layer(1, "h2", h1, h2, "h1", 1, 2, 144, 256, 12, 4,
                            pre=lambda: HOLD.__setitem__(
                                "h1", layer_stats("h1", stt0, 1, 72)))
        stt_d2 = down_layer(2, "h3", h2, h3, "h2", 2, 4, 72, 128, 12, 4,
                            pre=lambda: HOLD.__setitem__(
                                "h2", layer_stats("h2", stt_d1, 2, 18)))
        stt_d3 = down_layer(3, "h4", h3, h4, "h3", 4, 8, 36, 64, 18, 9,
                            slice_fn=slices_d3, single=True,
                            pre=lambda: HOLD.__setitem__(
                                "h3", layer_stats("h3", stt_d2, 4, 9)))

        # ================= up layers u0..u2 ================================
        nc.leave_named_scope("down", _sc, False)
        _sc = nc.enter_named_scope("up", False)[0]

        def up_layer(li, name, src, dst, in_name, cbi, cbo, Sin, Wi, idx_map,
                     single=False, pre=None):
            Mo = 128
            Wo = 2 * Wi
            Sout = 2 * Sin
            nstrip, subs, slfn = UP_SCHED[name]
            nro = Sout // nstrip  # out rows per strip
            nent = len(LAYER_ENTRIES[name])
            stt = stp.tile([P, cbo, nent, 6], F32, name=f"stu{li}",
                           tag="stats")
            slabs = [None] * nstrip
            srows = []
            for s in range(nstrip):
                y0 = s * nro
                i_lo = max((y0 - 1) // 2, 0)
                i_hi = min((y0 + nro) // 2 + 1, Sin)
                srows.append((i_lo, i_hi))
            rows_in = max(hi - lo for lo, hi in srows) + 1

            def load(s):
                i_lo, i_hi = srows[s]
                sl = sb.tile([P, cbi, rows_in, Wi], F8, name=f"slu{li}",
                             tag="inslabB" if single else "inslab",
                             bufs=1 if single else None)
                for cb in range(cbi):
                    nc.sync.dma_start(
                        out=sl[:, cb, 0:i_hi - i_lo, :],
                        in_=_ap(src[cb, 0:P, 0, 0], i_lo * Wi,
                                [[Wi, i_hi - i_lo], [1, Wi]]))
                slabs[s] = sl

            def relu(s):
                i_lo, i_hi = srows[s]
                sl = slabs[s]
                st_in = HOLD[in_name]
                for cb in range(cbi):
                    nc.scalar.activation(
                        out=sl[:, cb, 0:i_hi - i_lo, :],
                        in_=sl[:, cb, 0:i_hi - i_lo, :], func=AF.Relu,
                        bias=st_in[cb][:, 1:2], scale=st_in[cb][:, 0:1])
                if i_hi - i_lo < rows_in:
                    nc.vector.memset(sl[:, :, i_hi - i_lo:rows_in, :], 0.0)

            def compute(s, m, wt):
                i_lo, i_hi = srows[s]
                sl = slabs[s]
                y0 = s * nro
                osb = osl.tile([Mo, nro, Wo], F8, name=f"osu{li}",
                               tag="outslab")
                for a in range(2):
                    kys = [1] if a == 0 else [0, 2]
                    for b_ in range(2):
                        kxs = [1] if b_ == 0 else [2, 0]
                        k0 = 0
                        for su, rsub in enumerate(subs):
                            yb = y0 + a + 2 * k0
                            pt = ps.tile([Mo, rsub, Wi], F32, name=f"ptu{li}",
                                         tag="mm")
                            first = True
                            for kx in kxs:
                                j0 = (b_ + 1 - kx) // 2
                                trim = 1 if (kx == 0 and j0 == 1) else 0
                                n = Wi - 1 if trim else Wi
                                o = pt[:, :, 0:n] if trim else pt
                                for ky in kys:
                                    i_first = (yb + 1 - ky) // 2
                                    for cbp in range(cbi // 2):
                                        boff = (2 * cbp * rows_in * Wi
                                                + (i_first - i_lo) * Wi + j0)
                                        rhs = _ap(sl[:, 0, 0, 0], boff,
                                                  [[rows_in * Wi, 2],
                                                   [Wi, rsub], [1, n]])
                                        last = (kx == kxs[-1] and ky == kys[-1]
                                                and cbp == cbi // 2 - 1)
                                        nc.tensor.matmul(
                                            o,
                                            wt[:, 2 * cbp:2 * cbp + 2,
                                               ky, kx, :],
                                            rhs, start=first, stop=last,
                                            perf_mode=DRM)
                                        first = False
                            for (ei, (klo, khi)) in enumerate(slfn(s, su)):
                                nc.vector.bn_stats(
                                    out=stt[:, m, idx_map[(s, a, b_, su, ei)], :],
                                    in_=_ap(pt[:, 0, 0], klo * Wi,
                                            [[1, (khi - klo) * Wi]]))
                            oap = _ap(osb[:, 0, 0],
                                      (a + 2 * k0) * Wo + b_,
                                      [[2 * Wo, rsub], [2, Wi]])
                            nc.scalar.activation(out=oap, in_=pt,
                                                 func=AF.Copy)
                            k0 += rsub
                nc.sync.dma_start(
                    out=_ap(dst[m, 0:Mo, 0, 0], y0 * Wo, [[Wo, nro], [1, Wo]]),
                    in_=osb)

            for s in range(nstrip):
                load(s)
                if s == 0 and pre is not None:
                    pre()
                relu(s)
                for m in range(cbo):
                    wt = wpm.tile([P, cbi, 3, 3, Mo], F8, name=f"wtu{li}",
                                  tag="wup")
                    nc.sync.dma_start(out=wt, in_=wu[li][m])
                    compute(s, m, wt)
            return stt

        stt_u0 = up_layer(0, "g0", h4, g0, "h4", 8, 4, 18, 32, IDX_G0,
                          single=True,
                          pre=lambda: HOLD.__setitem__(
                              "h4", layer_stats("h4", stt_d3, 8, 4)))
        stt_u1 = up_layer(1, "g1", g0, g1, "g0", 4, 2, 36, 64, IDX_G1,
                          pre=lambda: HOLD.__setitem__(
                              "g0", layer_stats("g0", stt_u0, 4, 16)))
        stt_u2 = up_layer(2, "g2", g1, g2, "g1", 2, 1, 72, 128, IDX_G2,
                          pre=lambda: HOLD.__setitem__(
                              "g1", layer_stats("g1", stt_u1, 2, 32)))

        # ================= u3: 128 -> 64, M-packed into g3 =================
        wu3at = wsm.tile([128, 3, 128], BF16, name="wu3at")
        nc.sync.dma_start(out=wu3at, in_=wu3a[:, :, :])
        wu3bt = wsm.tile([128, 3, 64], BF16, name="wu3bt")
        nc.sync.dma_start(out=wu3bt, in_=wu3b[:, :, :])
        stt3 = stp.tile([P, 1, 144, 6], F32, name="stt3", tag="stats")
        NU3 = 6
        slabs3 = [None] * NU3

        def u3_load(s):
            q0 = s * 24
            i_hi = min(q0 + 25, 144)
            sl = sb.tile([P, 26, 256], F8, name="sl3", tag="inslab")
            nc.sync.dma_start(out=sl[:, 0:i_hi - q0, :],
                              in_=_ap(g2[0, 0:P, 0, 0], q0 * 256,
                                      [[256, i_hi - q0], [1, 256]]))
            slabs3[s] = (sl, i_hi - q0)

        def u3_relu(s):
            sl, nreal = slabs3[s]
            st_ = HOLD["g2"][0]
            cuts = [(0, 13), (13, nreal)] if s == 0 else [(0, nreal)]
            for (rl, rh) in cuts:
                nc.scalar.activation(out=sl[:, rl:rh, :],
                                     in_=sl[:, rl:rh, :],
                                     func=AF.Relu, bias=st_[:, 1:2],
                                     scale=st_[:, 0:1])
            if nreal < 26:
                nc.vector.memset(sl[:, nreal:26, :], 0.0)

        def u3_compute(s):
            sl, _n = slabs3[s]
            q0s = s * 24
            for blk in range(12):
                q0 = blk * 2  # local to slab
                pb0 = ps.tile([P, 2, 256], F32, name="pb0", tag="mm")
                pb1 = ps.tile([P, 2, 256], F32, name="pb1", tag="mm")
                # T1: full, start
                nc.tensor.matmul(pb0, wu3at[:, 0, :],
                                 _ap(sl[:, 0, 0], q0 * 256,
                                     [[256, 2], [1, 256]]),
                                 start=True, stop=False)
                # T2: rhs i=q+1, M 64..127
                nc.tensor.matmul(pb0[64:128, :, :], wu3bt[:, 0, :],
                                 _ap(sl[:, 0, 0], (q0 + 1) * 256,
                                     [[256, 2], [1, 256]]),
                                 start=False, stop=True)
                # T3: full, start
                nc.tensor.matmul(pb1, wu3at[:, 1, :],
                                 _ap(sl[:, 0, 0], q0 * 256,
                                     [[256, 2], [1, 256]]),
                                 start=True, stop=False)
                # T4: cols p+1, trim last
                nc.tensor.matmul(pb1[:, :, 0:255], wu3at[:, 2, :],
                                 _ap(sl[:, 0, 0], q0 * 256 + 1,
                                     [[256, 2], [1, 255]]),
                                 start=False, stop=False)
                # T5: rhs i=q+1 col p, M 64..127
                nc.tensor.matmul(pb1[64:128, :, :], wu3bt[:, 1, :],
                                 _ap(sl[:, 0, 0], (q0 + 1) * 256,
                                     [[256, 2], [1, 256]]),
                                 start=False, stop=False)
                # T6: rhs i=q+1 col p+1, M 64..127, trim
                nc.tensor.matmul(pb1[64:128, :, 0:255], wu3bt[:, 2, :],
                                 _ap(sl[:, 0, 0], (q0 + 1) * 256 + 1,
                                     [[256, 2], [1, 255]]),
                                 start=False, stop=True)
                eidx = (s * 12 + blk) * 2
                osb = osl.tile([P, 2, 518], F8, name="os3", tag="outslab")
                nc.scalar.activation(
                    out=_ap(osb[:, 0, 0], 3, [[518, 2], [2, 256]]), in_=pb0,
                    func=AF.Copy)
                nc.vector.tensor_copy(
                    out=_ap(osb[:, 0, 0], 4, [[518, 2], [2, 256]]), in_=pb1)
                nc.vector.bn_stats(out=stt3[:, 0, eidx, :],
                                   in_=pb0.rearrange("p a b -> p (a b)"))
                nc.vector.bn_stats(out=stt3[:, 0, eidx + 1, :],
                                   in_=pb1.rearrange("p a b -> p (a b)"))
                # g3 pair = q + 2 (pad4 top)
                nc.sync.dma_start(
                    out=_ap(g3[0:P, 0, 0], (q0s + q0 + 2) * 518,
                            [[518, 2], [1, 518]]),
                    in_=osb)
        for s in range(NU3 + 2):
            if s < NU3:
                u3_load(s)
            if s == 1:
                HOLD["g2"] = layer_stats("g2", stt_u2, 1, 84)
            if s >= 2:
                u3_compute(s - 2)
            if 1 <= s < NU3 + 1:
                u3_relu(s - 1)

        # --- g3 gutters: rows (reflect, partition-sliced) + cols ----------
        # padded row 1 <- 7, 2 <- 6, 3 <- 5 ; 292 <- 290, 293 <- 289, 294<-288
        for (d_, s_) in ((0, 8), (1, 7), (2, 6), (3, 5), (292, 290),
                         (293, 289), (294, 288), (295, 287)):
            dp, dj = divmod(d_, 2)
            sp, sj = divmod(s_, 2)
            nc.sync.dma_start(
                out=_ap(g3[dj * 64:(dj + 1) * 64, 0, 0], dp * 518, [[1, 518]]),
                in_=_ap(g3[sj * 64:(sj + 1) * 64, 0, 0], sp * 518, [[1, 518]]))
        # ================= final 7x7 + tanh + masked sum ===================
        nc.leave_named_scope("up", _sc, False)
        _sc = nc.enter_named_scope("final", False)[0]
        wfAt = wsm.tile([128, 7, 126], BF16, name="wfAt")
        nc.sync.dma_start(out=wfAt, in_=wfA[:, :, :])
        wfSt = wsm.tile([126, 7, 18], BF16, name="wfSt")
        nc.sync.dma_start(out=wfSt, in_=wfS[:, :, :])
        bft = nrm.tile([18, 1], F32, name="bft")
        nc.sync.dma_start(out=bft, in_=bfv[:, :])
        acc = nrm.tile([18, 96], F32, name="acc")
        nc.vector.memset(acc, 0.0)
        NF = 24  # strips of 12 output rows (2 y0-groups of 6)
        fslabs = [None] * NF

        def f_load(s):
            sl = sb.tile([128, 10, 518], F8, name="slF", tag="inslab")
            nc.sync.dma_start(out=sl, in_=_ap(g3[0:128, 0, 0], 6 * s * 518,
                                              [[518, 10], [1, 518]]))
            # reflect column gutters in SBUF (g3 cols 0..2/515..517 are junk)
            for k in range(3):
                nc.vector.tensor_copy(out=sl[:, :, k:k + 1],
                                      in_=sl[:, :, 6 - k:7 - k])
                nc.vector.tensor_copy(out=sl[:, :, 515 + k:516 + k],
                                      in_=sl[:, :, 513 - k:514 - k])
            mt = sb.tile([18, 4, 256], F32, name="mt", tag="mslab")
            nc.sync.dma_start(out=mt, in_=_dap(maskrep, s * 1024,
                                               [[48 * 512, 18], [1, 1024]]))
            fslabs[s] = (sl, mt)

        def f_relu(s):
            sl, _m = fslabs[s]
            st_ = HOLD["g3"][0]
            cuts = [(0, 7), (7, 10)] if s == 0 else [(0, 10)]
            for (rl, rh) in cuts:
                nc.scalar.activation(out=sl[:, rl:rh, :], in_=sl[:, rl:rh, :],
                                     func=AF.Relu,
                                     bias=st_[:, 1:2], scale=st_[:, 0:1])

        def f_compute(s):
            sl, mt = fslabs[s]
            for g in range(2):
                for hx in range(2):
                    ptA_f = psf.tile([128, 262], F32, name="ptA", tag="fa")
                    ptA = ptA_f[0:126, :]
                    for t in range(7):
                        rhs = _ap(sl[:, 0, 0], (3 * g + t) * 518 + hx * 256,
                                  [[1, 262]])
                        nc.tensor.matmul(ptA, wfAt[:, t, :], rhs,
                                         start=(t == 0), stop=(t == 6))
                    stg = osl.tile([126, 262], F8, name="stg", tag="fstg")
                    nc.vector.tensor_copy(out=stg, in_=ptA)
                    ptB_f = psf.tile([128, 256], F32, name="ptB", tag="fb")
                    ptB = ptB_f[0:18, :]
                    for dx in range(7):
                        nc.tensor.matmul(ptB, wfSt[:, dx, :],
                                         stg[:, dx:dx + 256],
                                         start=(dx == 0), stop=(dx == 6))
                    t1 = osl.tile([18, 256], F32, name="ft1", tag="ft1")
                    nc.vector.tensor_scalar(out=t1, in0=ptB, scalar1=bft,
                                            scalar2=None, op0=ALU.add)
                    nc.vector.tensor_mul(out=t1, in0=t1,
                                         in1=mt[:, 2 * g + hx, :])
                    th = osl.tile([18, 256], F32, name="fth", tag="fth")
                    ai = 4 * s + 2 * g + hx
                    nc.scalar.activation(out=th, in_=t1, func=AF.Tanh,
                                         accum_out=acc[:, ai:ai + 1])

        for s in range(NF + 2):
            if s < NF:
                f_load(s)
            if s == 1:
                HOLD["g3"] = layer_stats("g3", stt3, 1, 144, fold=True)
            if s >= 2:
                f_compute(s - 2)
            if 1 <= s < NF + 1:
                f_relu(s - 1)

        osum_t = nrm.tile([18, 1], F32, name="osum_t")
        nc.vector.tensor_reduce(out=osum_t, in_=acc, op=ALU.add,
                                axis=mybir.AxisListType.X)
        nc.sync.dma_start(out=osum[:, :], in_=osum_t)

        if debug:
            for nm, tens in [("h0", h0), ("g3", g3)]:
                sh = dbg[nm].shape
                nc.sync.dma_start(
                    out=_dap(dbg[nm], 0, [[sh[1], sh[0]], [1, sh[1]]]),
                    in_=_dap(tens, 0, [[sh[1], sh[0]], [1, sh[1]]]))
            for nm, tens, cbo, sz in [("h1", h1, 1, 144 * 256),
                                      ("h2", h2, 2, 72 * 128),
                                      ("h3", h3, 4, 36 * 64),
                                      ("h4", h4, 8, 18 * 32),
                                      ("g0", g0, 4, 36 * 64),
                                      ("g1", g1, 2, 72 * 128),
                                      ("g2", g2, 1, 144 * 256)]:
                for m in range(cbo):
                    nc.sync.dma_start(
                        out=_dap(dbg[nm], m * P * sz, [[sz, P], [1, sz]]),
                        in_=_dap(tens, m * P * sz, [[sz, P], [1, sz]]))
            for i, nm in enumerate(LAYER_ORDER):
                nc.sync.dma_start(out=dbg["st"][:, 2 * i:2 * i + 2],
                                  in_=HOLD[nm][0][:, :])
            off = 0
            for k in ["h0", "h1", "h2", "h3", "h4", "g0", "g1", "g2", "g3"]:
                n_ = CT[k]
                nc.sync.dma_start(out=dbg["sr"][off:off + n_, :],
                                  in_=stat_r[k][0:n_, :])
                nc.sync.dma_start(out=dbg["sl"][off:off + n_, :],
                                  in_=stat_l[k][:, :])
                off += n_

    nc.finalize()
    return nc


_CACHE = {}


def make_in_maps(inputs):
    wblobs = prep_weights(inputs)
    x = np.asarray(inputs["x"], np.float32)
    inst = np.asarray(inputs["inst"])
    return [prep_core_inputs(x[c // 2], inst[c // 2, 0], wblobs, c % 2)
            for c in range(8)]


def combine_outputs(res, inst):
    mask = (np.asarray(inst) == 1).astype(np.float32)  # [B,1,H,W]
    cnt = mask.sum((2, 3))  # [B,1]
    out = np.zeros((B, 3, H, W), np.float32)
    for b in range(B):
        s_top = res.results[2 * b]["osum"].reshape(6, 3).sum(0)
        s_bot = res.results[2 * b + 1]["osum"].reshape(6, 3).sum(0)
        mean = (s_top + s_bot) / cnt[b, 0]
        out[b] = mean[:, None, None] * mask[b, 0]
    return out


def kernel(**inputs):
    if "nc" not in _CACHE:
        _CACHE["nc"] = build_kernel()
    res = run_bass_kernel_spmd(_CACHE["nc"], make_in_maps(inputs),
                               core_ids=list(range(8)))
    return combine_outputs(res, inputs["inst"])

